# revision 12
# baseline (speedup 1.0000x reference)
"""GCN encoder (Linear+ReLU -> GCNConv+ReLU -> GCNConv -> ReLU) on 8 TRN2
NeuronCores.

Architecture (v8): fully node-sharded with SPLIT AllGathers and
segment-ordered edge tiles so the collective latency hides under gather
work.

  - Core c computes z1 = dinv*(relu(x_c @ fc_W + fc_b) @ W1) for its own
    2500 nodes, written as two DRAM halves m1A (rows 0-1279) and m1B
    (rows 1280-2499).  Two AllGathers (cc1a, cc1b) place the halves into
    disjoint regions of one shared table full1 [N, 256]:
      rows l <  1280:  full1[r*1280 + l]
      rows l >= 1280:  full1[10240 + r*1220 + (l-1280)]
  - Edges (dst-sharded) are packed per 128-dst chunk into four segments
    with core-uniform tile capacities:
      L0: src in own shard, l<1280  -> gather from m1A (no collective dep)
      L1: src in own shard, l>=1280 -> gather from m1B
      A : src l<1280 anywhere       -> gather from full1, waits cc1a
      B : src l>=1280               -> gather from full1, waits cc1b
    Overflow beyond a segment's capacity demotes the edge to a later
    segment (always legal); underflow pads with (row 0, w 0).
  - Aggregation runs as three psum passes (local incl self+bias / A / B)
    accumulating into f32 SBUF accumulators; the B pass finishes each
    chunk: relu, z2 = dinv*(l1 @ W2) production, AG2a/AG2b on the z2
    halves, then the same three-pass structure for layer 2.
  - The one-hot edge-weight matrices (lhsT of the aggregation matmuls)
    are built on device, one fused DVE tensor_scalar per tile:
    (iota_row == dslot) * w, from tiny [128, net] bf16 host arrays.
    No big selw input.
  - Degrees come from a single DVE reduce over a compact host layout of
    the own-shard edge weights (w at [dst%128, dst//128, k]).

Host-side preprocessing is index manipulation / data layout only.  All
arithmetic (degree sums, rsqrt, matmuls, aggregation) runs on device.
"""

import os

import numpy as np
import ml_dtypes

import concourse.bacc as bacc
import concourse.bass as bass
import concourse.mybir as mybir
import concourse.tile as tile
from concourse.bass_utils import run_bass_kernel_spmd
from concourse.masks import make_identity

F32 = mybir.dt.float32
BF16 = mybir.dt.bfloat16
I16 = mybir.dt.int16

N = 20000
E = 320000
IN_FT, HID1, HID2, OUT_FT = 256, 400, 200, 128
NCORES = 8
SHARD = N // NCORES            # 2500 nodes per core
NCH = (SHARD + 127) // 128     # 20 local dst chunks per core (last 68)
H1CH = 10                      # chunks in the A half
H1 = H1CH * 128                # 1280
H2 = SHARD - H1                # 1220
NA = NCORES * H1               # A-region rows in the full tables
TAB1_W = 256                   # padded row width of layer-1 gather table
TL0 = 2                        # local-segment tile capacities per chunk
TL1 = 2
AluOp = mybir.AluOpType
ActFn = mybir.ActivationFunctionType


def _cdiv(a, b):
    return (a + b - 1) // b


# --------------------------------------------------------------------------
# Host-side sharding / layout
# --------------------------------------------------------------------------

def _idx_layout(a):
    g = a.astype(np.int16).reshape(-1, 16).T.copy()
    return np.ascontiguousarray(np.tile(g, (8, 1)))


def _prep_edges(edge_index, edge_attr):
    """Partition edges by dst shard, pack per-chunk into [L0|L1|A|B]
    segments with core-uniform tile capacities.  Self loops are NOT in
    the edge lists (identity-stationary on the zself chunk rows)."""
    src = np.ascontiguousarray(edge_index[0]).astype(np.int64)
    dst = np.ascontiguousarray(edge_index[1]).astype(np.int64)
    w_all = np.ascontiguousarray(edge_attr).astype(np.float32)

    per_core = []
    cnt = np.zeros((NCORES, NCH, 4), np.int64)
    for c in range(NCORES):
        lo = c * SHARD
        m = (dst >= lo) & (dst < lo + SHARD)
        s = src[m]
        d = dst[m] - lo
        w = w_all[m]
        r = s // SHARD
        l = s % SHARD
        own = r == c
        inA = l < H1
        seg = np.where(own, np.where(inA, 0, 1), np.where(inA, 2, 3))
        ch = d >> 7
        for j in range(NCH):
            for q in range(4):
                cnt[c, j, q] = int(((ch == j) & (seg == q)).sum())
        per_core.append((s, d, w, r, l, seg, ch))

    # capacities per chunk (uniform across cores)
    TA = np.zeros(NCH, np.int64)
    TB = np.zeros(NCH, np.int64)
    for j in range(NCH):
        ovf0 = np.maximum(0, cnt[:, j, 0] - 128 * TL0)
        loadA = cnt[:, j, 2] + ovf0
        TA[j] = max(1, int(_cdiv(int(loadA.max()), 128)))
        ovf1 = np.maximum(0, cnt[:, j, 1] - 128 * TL1)
        loadB = cnt[:, j, 3] + ovf1
        TB[j] = max(1, int(_cdiv(int(loadB.max()), 128)))
    sumTA = int(TA.sum())
    sumTB = int(TB.sum())
    net = NCH * (TL0 + TL1) + sumTA + sumTB
    # global tile seq offsets
    offL0 = [TL0 * j for j in range(NCH)]
    offL1 = [NCH * TL0 + TL1 * j for j in range(NCH)]
    cumA = np.concatenate([[0], np.cumsum(TA)])
    cumB = np.concatenate([[0], np.cumsum(TB)])
    baseA = NCH * (TL0 + TL1)
    baseB = baseA + sumTA
    offA = [baseA + int(cumA[j]) for j in range(NCH)]
    offB = [baseB + int(cumB[j]) for j in range(NCH)]

    # per-core own-degree layout sized by the global max per-dst count
    K2 = 0
    for c in range(NCORES):
        lo = c * SHARD
        m = (dst >= lo) & (dst < lo + SHARD)
        dl = dst[m] - lo
        cc = np.zeros((NCH, 128), np.int64)
        np.add.at(cc, (dl >> 7, dl & 127), 1)
        K2 = max(K2, int(cc.max()))

    in_edges = []
    for c in range(NCORES):
        s, d, w, r, l, seg, ch = per_core[c]
        rowAB = np.where(l < H1, r * H1 + l, NA + r * H2 + (l - H1))
        erow = np.zeros(net * 128, np.int64)
        dslot = np.zeros(net * 128, np.int64)
        wslot = np.zeros(net * 128, np.float32)

        def fill(seq0, ntile, rows, dsl, ws):
            o = 128 * seq0
            k = len(rows)
            assert k <= 128 * ntile, (k, ntile)
            erow[o:o + k] = rows
            dslot[o:o + k] = dsl
            wslot[o:o + k] = ws

        for j in range(NCH):
            mj = ch == j
            s0 = mj & (seg == 0)
            s1 = mj & (seg == 1)
            s2 = mj & (seg == 2)
            s3 = mj & (seg == 3)
            # seg0: rows l into m1A; overflow -> A (rowAB)
            i0 = np.flatnonzero(s0)
            keep0, ov0 = i0[:128 * TL0], i0[128 * TL0:]
            fill(offL0[j], TL0, l[keep0], d[keep0] & 127, w[keep0])
            # seg1: rows l-H1 into m1B; overflow -> B
            i1 = np.flatnonzero(s1)
            keep1, ov1 = i1[:128 * TL1], i1[128 * TL1:]
            fill(offL1[j], TL1, l[keep1] - H1, d[keep1] & 127, w[keep1])
            # A: seg2 + seg0 overflow (rows into full1A)
            iA = np.concatenate([np.flatnonzero(s2), ov0])
            fill(offA[j], int(TA[j]), rowAB[iA], d[iA] & 127, w[iA])
            # B: seg3 + seg1 overflow (rows into full1B, offset by NA)
            iB = np.concatenate([np.flatnonzero(s3), ov1])
            fill(offB[j], int(TB[j]), rowAB[iB] - NA, d[iB] & 127,
                 w[iB])

        degw_own = np.zeros((128, NCH, K2), np.float32)
        lo = c * SHARD
        m = (dst >= lo) & (dst < lo + SHARD)
        dl_all = dst[m] - lo
        wl_all = w_all[m]
        kfill2 = np.zeros((NCH, 128), np.int64)
        lp, lc = dl_all & 127, dl_all >> 7
        for i in range(len(dl_all)):
            p, chn = int(lp[i]), int(lc[i])
            degw_own[p, chn, kfill2[chn, p]] = wl_all[i]
            kfill2[chn, p] += 1

        bf = ml_dtypes.bfloat16
        in_edges.append({
            "egidx": _idx_layout(erow),
            "dslotb": np.ascontiguousarray(
                dslot.reshape(net, 128).T).astype(bf),
            "wb": np.ascontiguousarray(
                wslot.reshape(net, 128).T).astype(bf),
            "degw_own": degw_own.reshape(128, -1),
        })
    meta = dict(TA=[int(x) for x in TA], TB=[int(x) for x in TB],
                offL0=offL0, offL1=offL1, offA=offA, offB=offB,
                net=net, K2=K2)
    return meta, in_edges


# --------------------------------------------------------------------------
# Device program
# --------------------------------------------------------------------------

def _fix_multiwait(nc):
    """This neuronxcc build only accepts ONE sync-wait on non-EventSemaphore
    instructions; bacc's splitter allows two on DMAs.  Move excess waits onto
    inserted EventSemaphore NOPs (2 waits each) preceding the instruction."""
    nev = 0
    for bb in nc.main_func.blocks:
        changed = False
        out = []
        for ins in bb.instructions:
            si = ins.sync_info
            waits = list(si.on_wait) if si and si.on_wait else []
            limit = 2 if isinstance(ins, mybir.InstEventSemaphore) else 1
            if len(waits) > limit:
                extra, keep = waits[:-limit], waits[-limit:]
                for i in range(0, len(extra), 2):
                    ev = mybir.InstEventSemaphore(
                        name=f"{ins.name}-evw{i}", ins=[], outs=[])
                    ev.engine = ins.engine
                    ev.sync_info = mybir.SyncInfo(
                        on_wait=extra[i:i + 2], on_update=[])
                    out.append(ev)
                    nev += 1
                si.on_wait = keep
                changed = True
            out.append(ins)
        if changed:
            bb.instructions = out
    return nev


def _dummy_out(nc, wpool, out_d):
    for j in range(NCH):
        cw = min(128, SHARD - 128 * j)
        o_sb = wpool.tile([128, OUT_FT], F32, tag="osb")
        nc.vector.memset(o_sb[:], 0.0)
        nc.sync.dma_start(out=out_d[128 * j:128 * j + cw, :],
                          in_=o_sb[:cw, :])


def build_nc(meta):
    stage = int(os.environ.get("K_STAGE", "500"))
    TA, TB = meta["TA"], meta["TB"]
    offL0, offL1 = meta["offL0"], meta["offL1"]
    offA, offB = meta["offA"], meta["offB"]
    net, K2 = meta["net"], meta["K2"]
    nc = bacc.Bacc("TRN2", target_bir_lowering=False, debug=False,
                   num_devices=NCORES, num_swdge_queues=4)

    xt_d = nc.dram_tensor("xt", [IN_FT, SHARD], BF16, kind="ExternalInput")
    egidx_d = nc.dram_tensor("egidx", [128, 8 * net], I16,
                             kind="ExternalInput")
    dslotb_d = nc.dram_tensor("dslotb", [128, net], BF16,
                              kind="ExternalInput")
    wb_d = nc.dram_tensor("wb", [128, net], BF16, kind="ExternalInput")
    iotab_d = nc.dram_tensor("iotab", [128, 16 * 128], BF16,
                             kind="ExternalInput")
    degwo_d = nc.dram_tensor("degw_own", [128, NCH * K2], F32,
                             kind="ExternalInput")
    fcw_d = nc.dram_tensor("fcw", [IN_FT, HID1], BF16,
                           kind="ExternalInput")
    fcb_d = nc.dram_tensor("fcb", [HID1, 1], F32, kind="ExternalInput")
    w1_d = nc.dram_tensor("w1", [HID1, HID2], BF16, kind="ExternalInput")
    b1_d = nc.dram_tensor("b1", [1, HID2], BF16, kind="ExternalInput")
    w2_d = nc.dram_tensor("w2", [HID2, OUT_FT], BF16,
                          kind="ExternalInput")
    b2_d = nc.dram_tensor("b2", [1, OUT_FT], BF16, kind="ExternalInput")
    out_d = nc.dram_tensor("out", [SHARD, OUT_FT], F32,
                           kind="ExternalOutput")

    n_fi = _cdiv(IN_FT, 128)     # 2
    n_fo = _cdiv(HID1, 128)      # 4 (128,128,128,16)
    n_k2 = _cdiv(HID2, 128)      # 2 (128,72)
    fo_sizes = [min(128, HID1 - 128 * i) for i in range(n_fo)]
    k2_sizes = [min(128, HID2 - 128 * i) for i in range(n_k2)]
    NSUB = 5
    SUB = SHARD // NSUB          # 500

    def _emit(tc, cpool, gpool, wpool, apool, psA, psB, psT, dpool):
        # ---------------- early inputs ----------------
        degwo_sb = cpool.tile([128, NCH * K2], F32)
        nc.sync.dma_start(out=degwo_sb[:], in_=degwo_d[:])
        egidx_sb = cpool.tile([128, 8 * net], I16)
        nc.sync.dma_start(out=egidx_sb[:], in_=egidx_d[:])
        dslot_sb = cpool.tile([128, net], BF16)
        nc.sync.dma_start(out=dslot_sb[:], in_=dslotb_d[:])
        wb_sb = cpool.tile([128, net], BF16)
        nc.sync.dma_start(out=wb_sb[:], in_=wb_d[:])
        iota_sb = cpool.tile([128, 16 * 128], BF16)
        nc.sync.dma_start(out=iota_sb[:], in_=iotab_d[:])
        fcb_sb = cpool.tile([128, n_fo], F32, name="fcb_sb")
        for i in range(n_fo):
            nc.sync.dma_start(
                out=fcb_sb[:fo_sizes[i], i:i + 1],
                in_=fcb_d[128 * i:128 * i + fo_sizes[i], :])

        fcw_sb = []
        for i in range(n_fi):
            t = cpool.tile([128, HID1], BF16, name=f"fcw{i}")
            nc.scalar.dma_start(out=t[:],
                                in_=fcw_d[128 * i:128 * (i + 1), :])
            fcw_sb.append(t)
        w1_sb = []
        for i in range(n_fo):
            t = cpool.tile([fo_sizes[i], HID2], BF16, name=f"w1_{i}")
            nc.scalar.dma_start(
                out=t[:], in_=w1_d[128 * i:128 * i + fo_sizes[i], :])
            w1_sb.append(t)
        w2_sb = []
        for i in range(n_k2):
            t = cpool.tile([k2_sizes[i], OUT_FT], BF16, name=f"w2_{i}")
            nc.scalar.dma_start(
                out=t[:], in_=w2_d[128 * i:128 * i + k2_sizes[i], :])
            w2_sb.append(t)
        b1_sb = cpool.tile([1, HID2], BF16)
        nc.scalar.dma_start(out=b1_sb[:], in_=b1_d[:])
        b2_sb = cpool.tile([1, OUT_FT], BF16)
        nc.scalar.dma_start(out=b2_sb[:], in_=b2_d[:])

        ident = cpool.tile([128, 128], BF16)
        make_identity(nc, ident[:])

        # ---------------- degrees / normalization (DVE) ---------------
        deg_own = cpool.tile([128, NCH], F32)
        nc.vector.tensor_reduce(
            out=deg_own[:],
            in_=degwo_sb[:].rearrange("p (c k) -> p c k", k=K2),
            axis=mybir.AxisListType.X, op=AluOp.add)
        nc.vector.tensor_scalar_add(deg_own[:], deg_own[:], 1.0)
        dinv_own = cpool.tile([128, NCH], F32)
        nc.vector.reciprocal(out=dinv_own[:], in_=deg_own[:])
        nc.scalar.activation(out=dinv_own[:], in_=dinv_own[:],
                             func=ActFn.Sqrt)
        sqd_own = cpool.tile([128, NCH], BF16)
        nc.scalar.activation(out=sqd_own[:], in_=deg_own[:],
                             func=ActFn.Sqrt)

        # DRAM halves of the per-core tables
        m1A = dpool.tile([H1, TAB1_W], BF16)
        m1B = dpool.tile([H2, TAB1_W], BF16)
        m2A = dpool.tile([H1, OUT_FT], BF16)
        m2B = dpool.tile([H2, OUT_FT], BF16)

        if stage < 10:
            _dummy_out(nc, wpool, out_d)
            return

        # ---------------- phase A: z1 (own shard) -> m1A/m1B ----------
        with tc.tile_pool(name="phA", bufs=1) as ppool:
            h0strip = []
            for i in range(n_fo):
                t_h = ppool.tile([fo_sizes[i], SHARD], BF16,
                                 name=f"h0strip{i}")
                h0strip.append(t_h)
            with tc.tile_pool(name="phAw", bufs=2) as tpool:
                nxt_ch = 0
                for s in range(NSUB):
                    xts = []
                    for k in range(n_fi):
                        xk = tpool.tile([128, SUB], BF16, tag="xts",
                                        name=f"xts{k}", bufs=3)
                        nc.scalar.dma_start(
                            out=xk[:],
                            in_=xt_d[128 * k:128 * (k + 1),
                                     SUB * s:SUB * (s + 1)])
                        xts.append(xk)
                    for i in range(n_fo):
                        ps_h = psA.tile([fo_sizes[i], SUB], F32, tag="ph")
                        for k in range(n_fi):
                            nc.tensor.matmul(
                                out=ps_h[:],
                                lhsT=fcw_sb[k][:, 128 * i:128 * i
                                               + fo_sizes[i]],
                                rhs=xts[k][:],
                                start=(k == 0), stop=(k == n_fi - 1),
                            )
                        nc.vector.tensor_scalar(
                            out=h0strip[i][:, SUB * s:SUB * (s + 1)],
                            in0=ps_h[:],
                            scalar1=fcb_sb[:fo_sizes[i], i:i + 1],
                            scalar2=0.0,
                            op0=AluOp.add, op1=AluOp.max,
                        )
                    end = SUB * (s + 1)
                    while (nxt_ch + 1) * 128 <= end or (
                            s == NSUB - 1 and nxt_ch < NCH):
                        ch = nxt_ch
                        nxt_ch += 1
                        cw = min(128, SHARD - 128 * ch)
                        ps_z = psB.tile([128, HID2], F32, tag="b")
                        for i in range(n_fo):
                            nc.tensor.matmul(
                                out=ps_z[:cw, :],
                                lhsT=h0strip[i][:, 128 * ch:128 * ch + cw],
                                rhs=w1_sb[i][:],
                                start=(i == 0), stop=(i == n_fo - 1),
                            )
                        zrow = tpool.tile([128, TAB1_W], BF16, tag="zrow",
                                          name="zrow", bufs=3)
                        nc.scalar.mul(out=zrow[:cw, :HID2],
                                      in_=ps_z[:cw, :],
                                      mul=dinv_own[:cw, ch:ch + 1])
                        if ch < H1CH:
                            nc.sync.dma_start(
                                out=m1A[128 * ch:128 * ch + cw, :],
                                in_=zrow[:cw, :])
                        else:
                            o = 128 * ch - H1
                            nc.sync.dma_start(
                                out=m1B[o:o + cw, :], in_=zrow[:cw, :])

        # ---------------- collectives: layer-1 table ------------------
        if stage < 12:
            _dummy_out(nc, wpool, out_d)
            return

        rg = [list(range(NCORES))]
        full1A = nc.dram_tensor("full1A", [NA, TAB1_W], BF16,
                                addr_space="Shared")
        full1B = nc.dram_tensor("full1B", [N - NA, TAB1_W], BF16,
                                addr_space="Shared")
        cc1a_i = nc.gpsimd.collective_compute(
            "AllGather", AluOp.bypass, replica_groups=rg,
            ins=[m1A.opt()], outs=[full1A.ap()[:]],
        )
        cc1a = [cc1a_i.ins]
        if stage < 13:
            _dummy_out(nc, wpool, out_d)
            return
        cc1b_i = nc.gpsimd.collective_compute(
            "AllGather", AluOp.bypass, replica_groups=rg,
            ins=[m1B.opt()], outs=[full1B.ap()[:]],
        )
        cc1b = [cc1b_i.ins]

        # deferred: sqd row layout (PE op; avoid head-of-line pre-phA)
        ps_tr = psT.tile([NCH, 128], BF16, tag="tr")
        nc.tensor.transpose(out=ps_tr[:], in_=sqd_own[:],
                            identity=ident[:])
        sqd_rows = cpool.tile([NCH, 128], BF16)
        nc.vector.tensor_copy(out=sqd_rows[:], in_=ps_tr[:])
        if stage < 135:
            _dummy_out(nc, wpool, out_d)
            return
        sqdT = cpool.tile([1, 128 * NCH], BF16)
        for j in range(NCH):
            nc.sync.dma_start(out=sqdT[:, 128 * j:128 * (j + 1)],
                              in_=sqd_rows[j:j + 1, :])

        if stage < 140:
            _dummy_out(nc, wpool, out_d)
            return

        # ---------------- gather emission helper ----------------------
        gq = [0]

        def emit_gathers(streams, tag, table, width, ccdeps, seqs, grain,
                         bufs):
            """Gather tiles seqs (consecutive) from table in instructions
            of `grain` tiles; returns list of (buf, pos_of_seq) lookup."""
            lookup = {}
            s0, s1 = seqs
            k = s0
            while k < s1:
                nt = min(grain, s1 - k)
                graw = gpool.tile([128, grain * width], BF16, tag=tag,
                                  name=f"g{tag}", bufs=bufs)
                sub = graw[:, :nt * width].rearrange(
                    "p (t f) -> p t f", f=width)
                if isinstance(table, bass.DRamTensorHandle):
                    table_ap = table.ap()
                else:
                    table_ap = table[:]
                gi = nc.gpsimd.dma_gather(
                    sub, table_ap, egidx_sb[:, 8 * k:8 * (k + nt)],
                    nt * 128, nt * 128, width, queue_num=gq[0] % 4)
                gq[0] += 1
                for cc in ccdeps:
                    tile.add_dep_helper(gi.ins, cc,
                                        reason="gather reads AG table")
                for t in range(nt):
                    lookup[k + t] = (graw, t)
                k += nt
            streams.update(lookup)

        # one-hot weight tiles: built on DVE in groups of 16 via
        # broadcast tensor_tensor; lazily, just before first use so the
        # DVE stream interleaves with the pass ops (release order).
        sw_groups = {}

        def swtile(seq, region0, region1):
            g0 = region0 + ((seq - region0) // 16) * 16
            if g0 not in sw_groups:
                nt = min(16, region1 - g0)
                swg = wpool.tile([128, 16 * 128], BF16, tag="sw",
                                 bufs=8)
                eq = wpool.tile([128, 16 * 128], BF16, tag="sweq",
                                bufs=1)
                e3 = eq[:, :nt * 128].rearrange("p (t f) -> p t f",
                                                f=128)
                nc.vector.tensor_tensor(
                    out=e3,
                    in0=iota_sb[:, :nt * 128].rearrange(
                        "p (t f) -> p t f", f=128),
                    in1=dslot_sb[:, g0:g0 + nt].broadcast_to(
                        [128, nt, 128]),
                    op=AluOp.is_equal)
                nc.vector.tensor_tensor(
                    out=swg[:, :nt * 128].rearrange(
                        "p (t f) -> p t f", f=128),
                    in0=e3,
                    in1=wb_sb[:, g0:g0 + nt].broadcast_to(
                        [128, nt, 128]),
                    op=AluOp.mult)
                sw_groups[g0] = swg
            return sw_groups[g0][:, (seq - g0) * 128:
                                 (seq - g0 + 1) * 128]

        # ---------------- layer-1 gathers (gpsimd order) --------------
        baseA = NCH * (TL0 + TL1)
        baseB = baseA + sum(TA)
        g1 = {}
        emit_gathers(g1, "g1L0", m1A, TAB1_W, [], (0, NCH * TL0), 8, 3)
        emit_gathers(g1, "g1L1", m1B, TAB1_W, [],
                     (NCH * TL0, NCH * (TL0 + TL1)), 8, 3)
        if stage >= 250:
            emit_gathers(g1, "g1A", full1A, TAB1_W, cc1a, (baseA, baseB),
                         8, 3)
        if stage >= 300:
            emit_gathers(g1, "g1B", full1B, TAB1_W, cc1b, (baseB, net),
                         8, 3)

        def agg_mm(ps, g, seq, width, start, stop, region):
            graw, t = g[seq]
            sw = swtile(seq, region[0], region[1])
            nc.tensor.matmul(
                out=ps[:],
                lhsT=sw,
                rhs=graw[:, t * width:t * width + (HID2 if width == TAB1_W
                                                   else width)],
                start=start, stop=stop,
            )

        # ---------------- layer-1 passes ------------------------------
        l1acc = apool.tile([128, NCH, HID2], F32)
        # local pass, L0 sub-pass (incl self + bias)
        for j in range(NCH):
            cw = min(128, SHARD - 128 * j)
            zself = wpool.tile([128, TAB1_W], BF16, tag="zself1", bufs=2)
            if j < H1CH:
                nc.sync.dma_start(out=zself[:cw, :],
                                  in_=m1A[128 * j:128 * j + cw, :])
            else:
                o = 128 * j - H1
                nc.sync.dma_start(out=zself[:cw, :],
                                  in_=m1B[o:o + cw, :])
            ps = psB.tile([128, HID2], F32, tag="b")
            for t in range(TL0):
                agg_mm(ps, g1, offL0[j] + t, TAB1_W, t == 0, False,
                       (0, NCH * TL0))
            nc.tensor.matmul(out=ps[:], lhsT=ident[:cw, :],
                             rhs=zself[:cw, :HID2],
                             start=False, stop=False)
            nc.tensor.matmul(out=ps[:],
                             lhsT=sqdT[:, 128 * j:128 * (j + 1)],
                             rhs=b1_sb[:], start=False, stop=True)
            nc.vector.tensor_copy(out=l1acc[:, j, :], in_=ps[:])
        # local pass, L1 sub-pass
        for j in range(NCH):
            ps = psB.tile([128, HID2], F32, tag="b")
            for t in range(TL1):
                agg_mm(ps, g1, offL1[j] + t, TAB1_W, t == 0,
                       t == TL1 - 1, (NCH * TL0, baseA))
            nc.vector.tensor_tensor(out=l1acc[:, j, :], in0=ps[:],
                                    in1=l1acc[:, j, :], op=AluOp.add)
        if stage < 250:
            _dummy_out(nc, wpool, out_d)
            return
        # A pass
        for j in range(NCH):
            ps = psB.tile([128, HID2], F32, tag="b")
            for t in range(TA[j]):
                agg_mm(ps, g1, offA[j] + t, TAB1_W, t == 0,
                       t == TA[j] - 1, (baseA, baseB))
            nc.vector.tensor_tensor(out=l1acc[:, j, :], in0=ps[:],
                                    in1=l1acc[:, j, :], op=AluOp.add)
        if stage < 300:
            _dummy_out(nc, wpool, out_d)
            return
        # B pass + z2 production
        for j in range(NCH):
            cw = min(128, SHARD - 128 * j)
            ps = psB.tile([128, HID2], F32, tag="b")
            for t in range(TB[j]):
                agg_mm(ps, g1, offB[j] + t, TAB1_W, t == 0,
                       t == TB[j] - 1, (baseB, net))
            acc = wpool.tile([128, HID2], F32, tag="l1f", bufs=2)
            nc.vector.tensor_tensor(out=acc[:], in0=ps[:],
                                    in1=l1acc[:, j, :], op=AluOp.add)
            l1row = wpool.tile([128, HID2], BF16, tag="l1r", bufs=2)
            nc.scalar.activation(out=l1row[:], in_=acc[:],
                                 func=ActFn.Relu,
                                 scale=dinv_own[:, j:j + 1])
            # ---- z2 for chunk j ----
            l1T = []
            for i in range(n_k2):
                ps_tr2 = psT.tile([128, 128], BF16, tag="tr")
                nc.tensor.transpose(
                    out=ps_tr2[:k2_sizes[i], :],
                    in_=l1row[:, 128 * i:128 * i + k2_sizes[i]],
                    identity=ident[:],
                )
                lt2 = wpool.tile([128, 128], BF16, tag="l1T")
                nc.vector.tensor_copy(out=lt2[:k2_sizes[i], :],
                                      in_=ps_tr2[:k2_sizes[i], :])
                l1T.append(lt2)
            ps_z2 = psB.tile([128, OUT_FT], F32, tag="b")
            for i in range(n_k2):
                nc.tensor.matmul(
                    out=ps_z2[:],
                    lhsT=l1T[i][:k2_sizes[i], :],
                    rhs=w2_sb[i][:],
                    start=(i == 0), stop=(i == n_k2 - 1),
                )
            zrow2 = wpool.tile([128, OUT_FT], BF16, tag="zrow2", bufs=3)
            nc.scalar.mul(out=zrow2[:], in_=ps_z2[:],
                          mul=dinv_own[:, j:j + 1])
            if j < H1CH:
                nc.sync.dma_start(out=m2A[128 * j:128 * j + cw, :],
                                  in_=zrow2[:cw, :])
            else:
                o = 128 * j - H1
                nc.sync.dma_start(out=m2B[o:o + cw, :],
                                  in_=zrow2[:cw, :])

        if stage < 400:
            for j in range(NCH):
                cw = min(128, SHARD - 128 * j)
                o_sb = wpool.tile([128, OUT_FT], F32, tag="osb")
                nc.scalar.copy(out=o_sb[:],
                               in_=l1acc[:, j, :OUT_FT])
                nc.sync.dma_start(out=out_d[128 * j:128 * j + cw, :],
                                  in_=o_sb[:cw, :])
            return

        # ---------------- collectives: layer-2 table ------------------
        full2A = nc.dram_tensor("full2A", [NA, OUT_FT], BF16,
                                addr_space="Shared")
        full2B = nc.dram_tensor("full2B", [N - NA, OUT_FT], BF16,
                                addr_space="Shared")
        cc2a_i = nc.gpsimd.collective_compute(
            "AllGather", AluOp.bypass, replica_groups=rg,
            ins=[m2A.opt()], outs=[full2A.ap()[:]],
        )
        cc2b_i = nc.gpsimd.collective_compute(
            "AllGather", AluOp.bypass, replica_groups=rg,
            ins=[m2B.opt()], outs=[full2B.ap()[:]],
        )
        cc2a, cc2b = [cc2a_i.ins], [cc2b_i.ins]

        if stage < 500:
            _dummy_out(nc, wpool, out_d)
            return

        # ---------------- layer-2 gathers + passes --------------------
        g2 = {}
        emit_gathers(g2, "g2L0", m2A, OUT_FT, [], (0, NCH * TL0), 8, 3)
        emit_gathers(g2, "g2L1", m2B, OUT_FT, [],
                     (NCH * TL0, NCH * (TL0 + TL1)), 8, 3)
        emit_gathers(g2, "g2A", full2A, OUT_FT, cc2a, (baseA, baseB),
                     8, 3)
        emit_gathers(g2, "g2B", full2B, OUT_FT, cc2b, (baseB, net),
                     8, 3)

        sw_groups.clear()
        l2acc = apool.tile([128, NCH, OUT_FT], BF16)
        for j in range(NCH):
            cw = min(128, SHARD - 128 * j)
            zself = wpool.tile([128, OUT_FT], BF16, tag="zself2", bufs=2)
            if j < H1CH:
                nc.sync.dma_start(out=zself[:cw, :],
                                  in_=m2A[128 * j:128 * j + cw, :])
            else:
                o = 128 * j - H1
                nc.sync.dma_start(out=zself[:cw, :],
                                  in_=m2B[o:o + cw, :])
            ps = psB.tile([128, OUT_FT], F32, tag="b")
            for t in range(TL0):
                agg_mm(ps, g2, offL0[j] + t, OUT_FT, t == 0, False,
                       (0, NCH * TL0))
            nc.tensor.matmul(out=ps[:], lhsT=ident[:cw, :],
                             rhs=zself[:cw, :],
                             start=False, stop=False)
            nc.tensor.matmul(out=ps[:],
                             lhsT=sqdT[:, 128 * j:128 * (j + 1)],
                             rhs=b2_sb[:], start=False, stop=True)
            nc.vector.tensor_copy(out=l2acc[:, j, :], in_=ps[:])
        for j in range(NCH):
            ps = psB.tile([128, OUT_FT], F32, tag="b")
            for t in range(TL1):
                agg_mm(ps, g2, offL1[j] + t, OUT_FT, t == 0,
                       t == TL1 - 1, (NCH * TL0, baseA))
            nc.vector.tensor_tensor(out=l2acc[:, j, :], in0=ps[:],
                                    in1=l2acc[:, j, :], op=AluOp.add)
        for j in range(NCH):
            ps = psB.tile([128, OUT_FT], F32, tag="b")
            for t in range(TA[j]):
                agg_mm(ps, g2, offA[j] + t, OUT_FT, t == 0,
                       t == TA[j] - 1, (baseA, baseB))
            nc.vector.tensor_tensor(out=l2acc[:, j, :], in0=ps[:],
                                    in1=l2acc[:, j, :], op=AluOp.add)
        for j in range(NCH):
            cw = min(128, SHARD - 128 * j)
            ps = psB.tile([128, OUT_FT], F32, tag="b")
            for t in range(TB[j]):
                agg_mm(ps, g2, offB[j] + t, OUT_FT, t == 0,
                       t == TB[j] - 1, (baseB, net))
            o_f32 = wpool.tile([128, OUT_FT], F32, tag="of")
            nc.vector.tensor_tensor(out=o_f32[:], in0=ps[:],
                                    in1=l2acc[:, j, :], op=AluOp.add)
            o_sb = wpool.tile([128, OUT_FT], F32, tag="osb")
            nc.scalar.activation(out=o_sb[:], in_=o_f32[:],
                                 func=ActFn.Relu,
                                 scale=dinv_own[:, j:j + 1])
            nc.sync.dma_start(out=out_d[128 * j:128 * j + cw, :],
                              in_=o_sb[:cw, :])

    with tile.TileContext(nc) as tc:
        with (
            tc.tile_pool(name="const", bufs=1) as cpool,
            tc.tile_pool(name="gath", bufs=1) as gpool,
            tc.tile_pool(name="work", bufs=2) as wpool,
            tc.tile_pool(name="acc", bufs=1) as apool,
            tc.tile_pool(name="psA", bufs=3, space="PSUM") as psA,
            tc.tile_pool(name="psB", bufs=3, space="PSUM") as psB,
            tc.tile_pool(name="psT", bufs=2, space="PSUM") as psT,
            tc.tile_pool(name="dram", bufs=1, space="DRAM") as dpool,
        ):
            _emit(tc, cpool, gpool, wpool, apool, psA, psB, psT, dpool)
    nc.compile()
    _fix_multiwait(nc)
    return nc


# --------------------------------------------------------------------------
# Entry point
# --------------------------------------------------------------------------

_NC_CACHE = {}


def kernel(x, edge_index, edge_attr, fc_W, fc_b, W1, b1, W2, b2,
           _trace=False):
    meta, in_edges = _prep_edges(edge_index, edge_attr)
    key = (tuple(meta["TA"]), tuple(meta["TB"]), meta["K2"])
    if key not in _NC_CACHE:
        _NC_CACHE[key] = build_nc(meta)
    nc = _NC_CACHE[key]

    x = np.asarray(x, np.float32)
    bf = ml_dtypes.bfloat16
    iotab = np.ascontiguousarray(
        np.tile(np.arange(128, dtype=np.float32), (128, 16))).astype(bf)
    shared = {
        "fcw": np.asarray(fc_W, np.float32).astype(bf),
        "fcb": np.asarray(fc_b, np.float32).reshape(HID1, 1),
        "w1": np.asarray(W1, np.float32).astype(bf),
        "b1": np.asarray(b1, np.float32).reshape(1, HID2).astype(bf),
        "w2": np.asarray(W2, np.float32).astype(bf),
        "b2": np.asarray(b2, np.float32).reshape(1, OUT_FT).astype(bf),
        "iotab": iotab,
    }
    in_maps = []
    for c in range(NCORES):
        xt = np.ascontiguousarray(
            x[c * SHARD:(c + 1) * SHARD, :].T).astype(bf)
        in_maps.append({"xt": xt, **in_edges[c], **shared})

    res = run_bass_kernel_spmd(nc, in_maps, list(range(NCORES)),
                               trace=_trace)
    out = np.concatenate([res.results[c]["out"] for c in range(NCORES)],
                         axis=0)
    if _trace:
        kernel._last_exec_time_ns = res.exec_time_ns
        kernel._last_results = res
    return out


# revision 13
# speedup vs baseline: 1.5000x; 1.5000x over previous
"""GCN encoder (Linear+ReLU -> GCNConv+ReLU -> GCNConv -> ReLU) on 8 TRN2
NeuronCores.

Architecture (v9): node-sharded, one AllGather per layer, with edge
tiles split [local | remote] so local work and table loads overlap the
collective latency.

  - Core c computes z1 = dinv*(relu(x_c @ fc_W + fc_b) @ W1) for its own
    2500 nodes into DRAM m1; AllGather concatenates shards into full1
    [N, 256] (row = global node id).
  - Edges (dst-sharded) are packed per 128-dst chunk into two segments
    with core-uniform tile capacities:
      L: src in own shard -> dma_gather from m1 (no collective dep)
      R: remote src       -> dma_gather from full1, waits cc1
    Local overflow beyond 2 tiles/chunk demotes to R (always legal);
    underflow pads with (row 0, w 0).
  - Aggregation runs as two psum passes per layer (local incl self+bias,
    then remote) accumulating via SBUF accumulators; the remote pass
    finishes each chunk: relu, z2 = dinv*(l1 @ W2), AG2, then the same
    two-pass structure for layer 2.
  - The one-hot edge-weight matrices (lhsT of the aggregation matmuls)
    are built on device in groups of 16 tiles with two broadcast DVE
    tensor_tensor ops: (iota == dslot_bcast) * w_bcast, from [128, net]
    bf16 host arrays.  No big selw input.
  - Gather/collective instructions are chained with scheduler-only
    ordering edges so the in-order GpSimd engine never blocks on a
    later collective while earlier gather work is pending.
  - Degrees come from a single DVE reduce over a compact host layout of
    the own-shard edge weights (w at [dst%128, dst//128, k]).

Host-side preprocessing is index manipulation / data layout only.  All
arithmetic (degree sums, rsqrt, matmuls, aggregation) runs on device.
"""

import os

import numpy as np
import ml_dtypes

import concourse.bacc as bacc
import concourse.bass as bass
import concourse.mybir as mybir
import concourse.tile as tile
from concourse.bass_utils import run_bass_kernel_spmd
from concourse.masks import make_identity

F32 = mybir.dt.float32
BF16 = mybir.dt.bfloat16
I16 = mybir.dt.int16

N = 20000
E = 320000
IN_FT, HID1, HID2, OUT_FT = 256, 400, 200, 128
NCORES = 8
SHARD = N // NCORES            # 2500 nodes per core
NCH = (SHARD + 127) // 128     # 20 local dst chunks per core (last 68)
TAB1_W = 256                   # padded row width of layer-1 gather table
TL = 2                         # local-segment tile capacity per chunk
AluOp = mybir.AluOpType
ActFn = mybir.ActivationFunctionType


def _cdiv(a, b):
    return (a + b - 1) // b


# --------------------------------------------------------------------------
# Host-side sharding / layout
# --------------------------------------------------------------------------

def _idx_layout(a):
    g = a.astype(np.int16).reshape(-1, 16).T.copy()
    return np.ascontiguousarray(np.tile(g, (8, 1)))


def _prep_edges(edge_index, edge_attr):
    """Partition edges by dst shard, pack per-chunk into [L|R] segments
    with core-uniform tile capacities.  Self loops are NOT in the edge
    lists (identity-stationary on the zself chunk rows)."""
    src = np.ascontiguousarray(edge_index[0]).astype(np.int64)
    dst = np.ascontiguousarray(edge_index[1]).astype(np.int64)
    w_all = np.ascontiguousarray(edge_attr).astype(np.float32)

    per_core = []
    cntL = np.zeros((NCORES, NCH), np.int64)
    cntR = np.zeros((NCORES, NCH), np.int64)
    for c in range(NCORES):
        lo = c * SHARD
        m = (dst >= lo) & (dst < lo + SHARD)
        s = src[m]
        d = dst[m] - lo
        w = w_all[m]
        own = (s // SHARD) == c
        ch = d >> 7
        for j in range(NCH):
            cntL[c, j] = int(((ch == j) & own).sum())
            cntR[c, j] = int(((ch == j) & ~own).sum())
        per_core.append((s, d, w, own, ch))

    TR = np.zeros(NCH, np.int64)
    for j in range(NCH):
        loadR = cntR[:, j] + np.maximum(0, cntL[:, j] - 128 * TL)
        TR[j] = max(1, int(_cdiv(int(loadR.max()), 128)))
    sumTR = int(TR.sum())
    net = NCH * TL + sumTR
    offL = [TL * j for j in range(NCH)]
    cumR = np.concatenate([[0], np.cumsum(TR)])
    baseR = NCH * TL
    offR = [baseR + int(cumR[j]) for j in range(NCH)]

    # per-core own-degree layout sized by the global max per-dst count
    K2 = 0
    for c in range(NCORES):
        lo = c * SHARD
        m = (dst >= lo) & (dst < lo + SHARD)
        dl = dst[m] - lo
        cc = np.zeros((NCH, 128), np.int64)
        np.add.at(cc, (dl >> 7, dl & 127), 1)
        K2 = max(K2, int(cc.max()))

    in_edges = []
    bf = ml_dtypes.bfloat16
    for c in range(NCORES):
        s, d, w, own, ch = per_core[c]
        erow = np.zeros(net * 128, np.int64)
        dslot = np.zeros(net * 128, np.int64)
        wslot = np.zeros(net * 128, np.float32)

        def fill(seq0, ntile, rows, dsl, ws):
            o = 128 * seq0
            k = len(rows)
            assert k <= 128 * ntile, (k, ntile)
            erow[o:o + k] = rows
            dslot[o:o + k] = dsl
            wslot[o:o + k] = ws

        for j in range(NCH):
            mj = ch == j
            iL = np.flatnonzero(mj & own)
            keepL, ovL = iL[:128 * TL], iL[128 * TL:]
            fill(offL[j], TL, s[keepL] % SHARD, d[keepL] & 127, w[keepL])
            iR = np.concatenate([np.flatnonzero(mj & ~own), ovL])
            fill(offR[j], int(TR[j]), s[iR], d[iR] & 127, w[iR])

        degw_own = np.zeros((128, NCH, K2), np.float32)
        lo = c * SHARD
        m = (dst >= lo) & (dst < lo + SHARD)
        dl_all = dst[m] - lo
        wl_all = w_all[m]
        kfill2 = np.zeros((NCH, 128), np.int64)
        lp, lc = dl_all & 127, dl_all >> 7
        for i in range(len(dl_all)):
            p, chn = int(lp[i]), int(lc[i])
            degw_own[p, chn, kfill2[chn, p]] = wl_all[i]
            kfill2[chn, p] += 1

        in_edges.append({
            "egidx": _idx_layout(erow),
            "dslotb": np.ascontiguousarray(
                dslot.reshape(net, 128).T).astype(bf),
            "wb": np.ascontiguousarray(
                wslot.reshape(net, 128).T).astype(bf),
            "degw_own": degw_own.reshape(128, -1),
        })
    meta = dict(TR=[int(x) for x in TR], offL=offL, offR=offR,
                net=net, K2=K2)
    return meta, in_edges


# --------------------------------------------------------------------------
# Device program
# --------------------------------------------------------------------------

def _fix_multiwait(nc):
    """This neuronxcc build only accepts ONE sync-wait on non-EventSemaphore
    instructions; bacc's splitter allows two on DMAs.  Move excess waits onto
    inserted EventSemaphore NOPs (2 waits each) preceding the instruction."""
    nev = 0
    for bb in nc.main_func.blocks:
        changed = False
        out = []
        for ins in bb.instructions:
            si = ins.sync_info
            waits = list(si.on_wait) if si and si.on_wait else []
            limit = 2 if isinstance(ins, mybir.InstEventSemaphore) else 1
            if len(waits) > limit:
                extra, keep = waits[:-limit], waits[-limit:]
                for i in range(0, len(extra), 2):
                    ev = mybir.InstEventSemaphore(
                        name=f"{ins.name}-evw{i}", ins=[], outs=[])
                    ev.engine = ins.engine
                    ev.sync_info = mybir.SyncInfo(
                        on_wait=extra[i:i + 2], on_update=[])
                    out.append(ev)
                    nev += 1
                si.on_wait = keep
                changed = True
            out.append(ins)
        if changed:
            bb.instructions = out
    return nev


def _dummy_out(nc, wpool, out_d):
    for j in range(NCH):
        cw = min(128, SHARD - 128 * j)
        o_sb = wpool.tile([128, OUT_FT], F32, tag="osb")
        nc.vector.memset(o_sb[:], 0.0)
        nc.sync.dma_start(out=out_d[128 * j:128 * j + cw, :],
                          in_=o_sb[:cw, :])


def build_nc(meta):
    stage = int(os.environ.get("K_STAGE", "500"))
    TR = meta["TR"]
    offL, offR = meta["offL"], meta["offR"]
    net, K2 = meta["net"], meta["K2"]
    nc = bacc.Bacc("TRN2", target_bir_lowering=False, debug=False,
                   num_devices=NCORES, num_swdge_queues=4)

    xt_d = nc.dram_tensor("xt", [IN_FT, SHARD], BF16, kind="ExternalInput")
    egidx_d = nc.dram_tensor("egidx", [128, 8 * net], I16,
                             kind="ExternalInput")
    dslotb_d = nc.dram_tensor("dslotb", [128, net], BF16,
                              kind="ExternalInput")
    wb_d = nc.dram_tensor("wb", [128, net], BF16, kind="ExternalInput")
    iotab_d = nc.dram_tensor("iotab", [128, 16 * 128], BF16,
                             kind="ExternalInput")
    degwo_d = nc.dram_tensor("degw_own", [128, NCH * K2], F32,
                             kind="ExternalInput")
    fcw_d = nc.dram_tensor("fcw", [IN_FT, HID1], BF16,
                           kind="ExternalInput")
    fcb_d = nc.dram_tensor("fcb", [HID1, 1], F32, kind="ExternalInput")
    w1_d = nc.dram_tensor("w1", [HID1, HID2], BF16, kind="ExternalInput")
    b1_d = nc.dram_tensor("b1", [1, HID2], BF16, kind="ExternalInput")
    w2_d = nc.dram_tensor("w2", [HID2, OUT_FT], BF16,
                          kind="ExternalInput")
    b2_d = nc.dram_tensor("b2", [1, OUT_FT], BF16, kind="ExternalInput")
    out_d = nc.dram_tensor("out", [SHARD, OUT_FT], F32,
                           kind="ExternalOutput")

    n_fi = _cdiv(IN_FT, 128)     # 2
    n_fo = _cdiv(HID1, 128)      # 4 (128,128,128,16)
    n_k2 = _cdiv(HID2, 128)      # 2 (128,72)
    fo_sizes = [min(128, HID1 - 128 * i) for i in range(n_fo)]
    k2_sizes = [min(128, HID2 - 128 * i) for i in range(n_k2)]
    NSUB = 5
    SUB = SHARD // NSUB          # 500

    def _emit(tc, cpool, gpool, wpool, apool, psA, psB, psT, dpool):
        # ---------------- early inputs ----------------
        degwo_sb = cpool.tile([128, NCH * K2], F32)
        nc.sync.dma_start(out=degwo_sb[:], in_=degwo_d[:])
        egidx_sb = cpool.tile([128, 8 * net], I16)
        nc.sync.dma_start(out=egidx_sb[:], in_=egidx_d[:])
        dslot_sb = cpool.tile([128, net], BF16)
        nc.sync.dma_start(out=dslot_sb[:], in_=dslotb_d[:])
        wb_sb = cpool.tile([128, net], BF16)
        nc.sync.dma_start(out=wb_sb[:], in_=wb_d[:])
        iota_sb = cpool.tile([128, 16 * 128], BF16)
        nc.sync.dma_start(out=iota_sb[:], in_=iotab_d[:])
        fcb_sb = cpool.tile([128, n_fo], F32, name="fcb_sb")
        for i in range(n_fo):
            nc.sync.dma_start(
                out=fcb_sb[:fo_sizes[i], i:i + 1],
                in_=fcb_d[128 * i:128 * i + fo_sizes[i], :])

        fcw_sb = []
        for i in range(n_fi):
            t = cpool.tile([128, HID1], BF16, name=f"fcw{i}")
            nc.scalar.dma_start(out=t[:],
                                in_=fcw_d[128 * i:128 * (i + 1), :])
            fcw_sb.append(t)
        w1_sb = []
        for i in range(n_fo):
            t = cpool.tile([fo_sizes[i], HID2], BF16, name=f"w1_{i}")
            nc.scalar.dma_start(
                out=t[:], in_=w1_d[128 * i:128 * i + fo_sizes[i], :])
            w1_sb.append(t)
        w2_sb = []
        for i in range(n_k2):
            t = cpool.tile([k2_sizes[i], OUT_FT], BF16, name=f"w2_{i}")
            nc.scalar.dma_start(
                out=t[:], in_=w2_d[128 * i:128 * i + k2_sizes[i], :])
            w2_sb.append(t)
        b1_sb = cpool.tile([1, HID2], BF16)
        nc.scalar.dma_start(out=b1_sb[:], in_=b1_d[:])
        b2_sb = cpool.tile([1, OUT_FT], BF16)
        nc.scalar.dma_start(out=b2_sb[:], in_=b2_d[:])

        ident = cpool.tile([128, 128], BF16)
        make_identity(nc, ident[:])

        # ---------------- degrees / normalization (DVE) ---------------
        deg_own = cpool.tile([128, NCH], F32)
        nc.vector.tensor_reduce(
            out=deg_own[:],
            in_=degwo_sb[:].rearrange("p (c k) -> p c k", k=K2),
            axis=mybir.AxisListType.X, op=AluOp.add)
        nc.vector.tensor_scalar_add(deg_own[:], deg_own[:], 1.0)
        dinv_own = cpool.tile([128, NCH], F32)
        nc.vector.reciprocal(out=dinv_own[:], in_=deg_own[:])
        nc.scalar.activation(out=dinv_own[:], in_=dinv_own[:],
                             func=ActFn.Sqrt)
        sqd_own = cpool.tile([128, NCH], BF16)
        nc.scalar.activation(out=sqd_own[:], in_=deg_own[:],
                             func=ActFn.Sqrt)

        m1 = dpool.tile([SHARD, TAB1_W], BF16)
        m2 = dpool.tile([SHARD, OUT_FT], BF16)

        if stage < 10:
            _dummy_out(nc, wpool, out_d)
            return

        # ---------------- phase A: z1 (own shard) -> m1 ---------------
        with tc.tile_pool(name="phA", bufs=1) as ppool:
            h0strip = []
            for i in range(n_fo):
                t_h = ppool.tile([fo_sizes[i], SHARD], BF16,
                                 name=f"h0strip{i}")
                h0strip.append(t_h)
            with tc.tile_pool(name="phAw", bufs=2) as tpool:
                nxt_ch = 0
                for s in range(NSUB):
                    xts = []
                    for k in range(n_fi):
                        xk = tpool.tile([128, SUB], BF16, tag="xts",
                                        name=f"xts{k}", bufs=3)
                        nc.scalar.dma_start(
                            out=xk[:],
                            in_=xt_d[128 * k:128 * (k + 1),
                                     SUB * s:SUB * (s + 1)])
                        xts.append(xk)
                    for i in range(n_fo):
                        ps_h = psA.tile([fo_sizes[i], SUB], F32, tag="ph")
                        for k in range(n_fi):
                            nc.tensor.matmul(
                                out=ps_h[:],
                                lhsT=fcw_sb[k][:, 128 * i:128 * i
                                               + fo_sizes[i]],
                                rhs=xts[k][:],
                                start=(k == 0), stop=(k == n_fi - 1),
                            )
                        nc.vector.tensor_scalar(
                            out=h0strip[i][:, SUB * s:SUB * (s + 1)],
                            in0=ps_h[:],
                            scalar1=fcb_sb[:fo_sizes[i], i:i + 1],
                            scalar2=0.0,
                            op0=AluOp.add, op1=AluOp.max,
                        )
                    end = SUB * (s + 1)
                    while (nxt_ch + 1) * 128 <= end or (
                            s == NSUB - 1 and nxt_ch < NCH):
                        ch = nxt_ch
                        nxt_ch += 1
                        cw = min(128, SHARD - 128 * ch)
                        ps_z = psB.tile([128, HID2], F32, tag="b")
                        for i in range(n_fo):
                            nc.tensor.matmul(
                                out=ps_z[:cw, :],
                                lhsT=h0strip[i][:, 128 * ch:128 * ch + cw],
                                rhs=w1_sb[i][:],
                                start=(i == 0), stop=(i == n_fo - 1),
                            )
                        zrow = tpool.tile([128, TAB1_W], BF16, tag="zrow",
                                          name="zrow", bufs=3)
                        nc.scalar.mul(out=zrow[:cw, :HID2],
                                      in_=ps_z[:cw, :],
                                      mul=dinv_own[:cw, ch:ch + 1])
                        nc.sync.dma_start(
                            out=m1[128 * ch:128 * ch + cw, :],
                            in_=zrow[:cw, :])

        if stage < 12:
            _dummy_out(nc, wpool, out_d)
            return

        # ---------------- collective: layer-1 table -------------------
        rg = [list(range(NCORES))]
        full1 = nc.dram_tensor("full1", [N, TAB1_W], BF16,
                               addr_space="Shared")
        cc1_i = nc.gpsimd.collective_compute(
            "AllGather", AluOp.bypass, replica_groups=rg,
            ins=[m1.opt()], outs=[full1.ap()[:]],
        )
        cc1 = [cc1_i.ins]
        gps_chain = [cc1_i.ins]

        # deferred: sqd row layout (PE op; avoid head-of-line pre-phA)
        ps_tr = psT.tile([NCH, 128], BF16, tag="tr")
        nc.tensor.transpose(out=ps_tr[:], in_=sqd_own[:],
                            identity=ident[:])
        sqd_rows = cpool.tile([NCH, 128], BF16)
        nc.vector.tensor_copy(out=sqd_rows[:], in_=ps_tr[:])
        sqdT = cpool.tile([1, 128 * NCH], BF16)
        for j in range(NCH):
            nc.sync.dma_start(out=sqdT[:, 128 * j:128 * (j + 1)],
                              in_=sqd_rows[j:j + 1, :])

        if stage < 14:
            _dummy_out(nc, wpool, out_d)
            return

        # ---------------- gather emission helper ----------------------
        gq = [0]

        def emit_gathers(streams, tag, table, width, ccdeps, seqs, grain,
                         bufs):
            s0, s1 = seqs
            k = s0
            while k < s1:
                nt = min(grain, s1 - k)
                graw = gpool.tile([128, grain * width], BF16, tag=tag,
                                  name=f"g{tag}", bufs=bufs)
                sub = graw[:, :nt * width].rearrange(
                    "p (t f) -> p t f", f=width)
                if isinstance(table, bass.DRamTensorHandle):
                    table_ap = table.ap()
                else:
                    table_ap = table[:]
                gi = nc.gpsimd.dma_gather(
                    sub, table_ap, egidx_sb[:, 8 * k:8 * (k + nt)],
                    nt * 128, nt * 128, width, queue_num=gq[0] % 4)
                gq[0] += 1
                for cc in ccdeps:
                    tile.add_dep_helper(gi.ins, cc,
                                        reason="gather reads AG table")
                if gps_chain:
                    tile.add_dep_helper(gi.ins, gps_chain[-1], sync=False,
                                        reason="gpsimd issue order")
                gps_chain.append(gi.ins)
                for t in range(nt):
                    streams[k + t] = (graw, t)
                k += nt

        # one-hot weight tiles: built on DVE in groups of 16 via
        # broadcast tensor_tensor; lazily, just before first use so the
        # DVE stream interleaves with the pass ops (release order).
        sw_groups = {}

        def swtile(seq, region0, region1):
            g0 = region0 + ((seq - region0) // 16) * 16
            if g0 not in sw_groups:
                nt = min(16, region1 - g0)
                swg = wpool.tile([128, 16 * 128], BF16, tag="sw",
                                 bufs=8)
                eq = wpool.tile([128, 16 * 128], BF16, tag="sweq",
                                bufs=1)
                e3 = eq[:, :nt * 128].rearrange("p (t f) -> p t f",
                                                f=128)
                nc.vector.tensor_tensor(
                    out=e3,
                    in0=iota_sb[:, :nt * 128].rearrange(
                        "p (t f) -> p t f", f=128),
                    in1=dslot_sb[:, g0:g0 + nt].broadcast_to(
                        [128, nt, 128]),
                    op=AluOp.is_equal)
                nc.vector.tensor_tensor(
                    out=swg[:, :nt * 128].rearrange(
                        "p (t f) -> p t f", f=128),
                    in0=e3,
                    in1=wb_sb[:, g0:g0 + nt].broadcast_to(
                        [128, nt, 128]),
                    op=AluOp.mult)
                sw_groups[g0] = swg
            return sw_groups[g0][:, (seq - g0) * 128:
                                 (seq - g0 + 1) * 128]

        def agg_mm(ps, g, seq, width, start, stop, region):
            graw, t = g[seq]
            sw = swtile(seq, region[0], region[1])
            nc.tensor.matmul(
                out=ps[:],
                lhsT=sw,
                rhs=graw[:, t * width:t * width + (HID2 if width == TAB1_W
                                                   else width)],
                start=start, stop=stop,
            )

        baseR = NCH * TL

        # ---------------- layer-1 gathers (gpsimd order) --------------
        g1 = {}
        emit_gathers(g1, "g1L", m1, TAB1_W, [], (0, baseR), 8, 3)
        if stage >= 250:
            emit_gathers(g1, "g1R", full1, TAB1_W, cc1, (baseR, net),
                         8, 6)

        # ---------------- layer-1 passes ------------------------------
        l1acc = apool.tile([128, NCH, HID2], F32)
        # local pass: L + self + bias
        for j in range(NCH):
            cw = min(128, SHARD - 128 * j)
            zself = wpool.tile([128, TAB1_W], BF16, tag="zself1", bufs=2)
            nc.sync.dma_start(out=zself[:cw, :],
                              in_=m1[128 * j:128 * j + cw, :])
            ps = psB.tile([128, HID2], F32, tag="b")
            for t in range(TL):
                agg_mm(ps, g1, offL[j] + t, TAB1_W, t == 0, False,
                       (0, baseR))
            nc.tensor.matmul(out=ps[:], lhsT=ident[:cw, :],
                             rhs=zself[:cw, :HID2],
                             start=False, stop=False)
            nc.tensor.matmul(out=ps[:],
                             lhsT=sqdT[:, 128 * j:128 * (j + 1)],
                             rhs=b1_sb[:], start=False, stop=True)
            nc.vector.tensor_copy(out=l1acc[:, j, :], in_=ps[:])

        if stage < 250:
            _dummy_out(nc, wpool, out_d)
            return

        # remote pass + z2 production
        for j in range(NCH):
            cw = min(128, SHARD - 128 * j)
            ps = psB.tile([128, HID2], F32, tag="b")
            for t in range(TR[j]):
                agg_mm(ps, g1, offR[j] + t, TAB1_W, t == 0,
                       t == TR[j] - 1, (baseR, net))
            acc = wpool.tile([128, HID2], F32, tag="l1f", bufs=2)
            nc.vector.tensor_tensor(out=acc[:], in0=ps[:],
                                    in1=l1acc[:, j, :], op=AluOp.add)
            l1row = wpool.tile([128, HID2], BF16, tag="l1r", bufs=2)
            nc.scalar.activation(out=l1row[:], in_=acc[:],
                                 func=ActFn.Relu,
                                 scale=dinv_own[:, j:j + 1])
            # ---- z2 for chunk j ----
            l1T = []
            for i in range(n_k2):
                ps_tr2 = psT.tile([128, 128], BF16, tag="tr")
                nc.tensor.transpose(
                    out=ps_tr2[:k2_sizes[i], :],
                    in_=l1row[:, 128 * i:128 * i + k2_sizes[i]],
                    identity=ident[:],
                )
                lt2 = wpool.tile([128, 128], BF16, tag="l1T")
                nc.vector.tensor_copy(out=lt2[:k2_sizes[i], :],
                                      in_=ps_tr2[:k2_sizes[i], :])
                l1T.append(lt2)
            ps_z2 = psB.tile([128, OUT_FT], F32, tag="b")
            for i in range(n_k2):
                nc.tensor.matmul(
                    out=ps_z2[:],
                    lhsT=l1T[i][:k2_sizes[i], :],
                    rhs=w2_sb[i][:],
                    start=(i == 0), stop=(i == n_k2 - 1),
                )
            zrow2 = wpool.tile([128, OUT_FT], BF16, tag="zrow2", bufs=3)
            nc.scalar.mul(out=zrow2[:], in_=ps_z2[:],
                          mul=dinv_own[:, j:j + 1])
            nc.sync.dma_start(out=m2[128 * j:128 * j + cw, :],
                              in_=zrow2[:cw, :])

        if stage < 400:
            for j in range(NCH):
                cw = min(128, SHARD - 128 * j)
                o_sb = wpool.tile([128, OUT_FT], F32, tag="osb")
                nc.scalar.copy(out=o_sb[:],
                               in_=l1acc[:, j, :OUT_FT])
                nc.sync.dma_start(out=out_d[128 * j:128 * j + cw, :],
                                  in_=o_sb[:cw, :])
            return

        # ---------------- collective: layer-2 table -------------------
        full2 = nc.dram_tensor("full2", [N, OUT_FT], BF16,
                               addr_space="Shared")
        cc2_i = nc.gpsimd.collective_compute(
            "AllGather", AluOp.bypass, replica_groups=rg,
            ins=[m2.opt()], outs=[full2.ap()[:]],
        )
        cc2 = [cc2_i.ins]
        tile.add_dep_helper(cc2_i.ins, gps_chain[-1], sync=False,
                            reason="gpsimd issue order")
        gps_chain.append(cc2_i.ins)

        if stage < 500:
            _dummy_out(nc, wpool, out_d)
            return

        # ---------------- layer-2 gathers + passes --------------------
        g2 = {}
        emit_gathers(g2, "g2L", m2, OUT_FT, [], (0, baseR), 8, 3)
        emit_gathers(g2, "g2R", full2, OUT_FT, cc2, (baseR, net), 8, 6)

        sw_groups.clear()
        l2acc = apool.tile([128, NCH, OUT_FT], BF16)
        for j in range(NCH):
            cw = min(128, SHARD - 128 * j)
            zself = wpool.tile([128, OUT_FT], BF16, tag="zself2", bufs=2)
            nc.sync.dma_start(out=zself[:cw, :],
                              in_=m2[128 * j:128 * j + cw, :])
            ps = psB.tile([128, OUT_FT], F32, tag="b")
            for t in range(TL):
                agg_mm(ps, g2, offL[j] + t, OUT_FT, t == 0, False,
                       (0, baseR))
            nc.tensor.matmul(out=ps[:], lhsT=ident[:cw, :],
                             rhs=zself[:cw, :],
                             start=False, stop=False)
            nc.tensor.matmul(out=ps[:],
                             lhsT=sqdT[:, 128 * j:128 * (j + 1)],
                             rhs=b2_sb[:], start=False, stop=True)
            nc.vector.tensor_copy(out=l2acc[:, j, :], in_=ps[:])
        for j in range(NCH):
            cw = min(128, SHARD - 128 * j)
            ps = psB.tile([128, OUT_FT], F32, tag="b")
            for t in range(TR[j]):
                agg_mm(ps, g2, offR[j] + t, OUT_FT, t == 0,
                       t == TR[j] - 1, (baseR, net))
            o_f32 = wpool.tile([128, OUT_FT], F32, tag="of")
            nc.vector.tensor_tensor(out=o_f32[:], in0=ps[:],
                                    in1=l2acc[:, j, :], op=AluOp.add)
            o_sb = wpool.tile([128, OUT_FT], F32, tag="osb")
            nc.scalar.activation(out=o_sb[:], in_=o_f32[:],
                                 func=ActFn.Relu,
                                 scale=dinv_own[:, j:j + 1])
            nc.sync.dma_start(out=out_d[128 * j:128 * j + cw, :],
                              in_=o_sb[:cw, :])

    with tile.TileContext(nc) as tc:
        with (
            tc.tile_pool(name="const", bufs=1) as cpool,
            tc.tile_pool(name="gath", bufs=1) as gpool,
            tc.tile_pool(name="work", bufs=2) as wpool,
            tc.tile_pool(name="acc", bufs=1) as apool,
            tc.tile_pool(name="psA", bufs=3, space="PSUM") as psA,
            tc.tile_pool(name="psB", bufs=3, space="PSUM") as psB,
            tc.tile_pool(name="psT", bufs=2, space="PSUM") as psT,
            tc.tile_pool(name="dram", bufs=1, space="DRAM") as dpool,
        ):
            _emit(tc, cpool, gpool, wpool, apool, psA, psB, psT, dpool)
    nc.compile()
    _fix_multiwait(nc)
    return nc


# --------------------------------------------------------------------------
# Entry point
# --------------------------------------------------------------------------

_NC_CACHE = {}


def kernel(x, edge_index, edge_attr, fc_W, fc_b, W1, b1, W2, b2,
           _trace=False):
    meta, in_edges = _prep_edges(edge_index, edge_attr)
    key = (tuple(meta["TR"]), meta["K2"])
    if key not in _NC_CACHE:
        _NC_CACHE[key] = build_nc(meta)
    nc = _NC_CACHE[key]

    x = np.asarray(x, np.float32)
    bf = ml_dtypes.bfloat16
    iotab = np.ascontiguousarray(
        np.tile(np.arange(128, dtype=np.float32), (128, 16))).astype(bf)
    shared = {
        "fcw": np.asarray(fc_W, np.float32).astype(bf),
        "fcb": np.asarray(fc_b, np.float32).reshape(HID1, 1),
        "w1": np.asarray(W1, np.float32).astype(bf),
        "b1": np.asarray(b1, np.float32).reshape(1, HID2).astype(bf),
        "w2": np.asarray(W2, np.float32).astype(bf),
        "b2": np.asarray(b2, np.float32).reshape(1, OUT_FT).astype(bf),
        "iotab": iotab,
    }
    in_maps = []
    for c in range(NCORES):
        xt = np.ascontiguousarray(
            x[c * SHARD:(c + 1) * SHARD, :].T).astype(bf)
        in_maps.append({"xt": xt, **in_edges[c], **shared})

    res = run_bass_kernel_spmd(nc, in_maps, list(range(NCORES)),
                               trace=_trace)
    out = np.concatenate([res.results[c]["out"] for c in range(NCORES)],
                         axis=0)
    if _trace:
        kernel._last_exec_time_ns = res.exec_time_ns
        kernel._last_results = res
    return out


# revision 15
# speedup vs baseline: 1.6369x; 1.0912x over previous
"""GCN encoder (Linear+ReLU -> GCNConv+ReLU -> GCNConv -> ReLU) on 8 TRN2
NeuronCores.

Architecture (v9.1): node-sharded; one AllGather for the layer-1 table,
a SPLIT AllGather (two collectives into disjoint slices of one shared
tensor) for the layer-2 table so layer-2 remote work starts while
layer-1 is still finishing.

  - Core c computes z1 = dinv*(relu(x_c @ fc_W + fc_b) @ W1) for its own
    2500 nodes into DRAM m1; AllGather cc1 concatenates shards into
    full1 [N, 256] (row = global node id).
  - Edges (dst-sharded) are packed per 128-dst chunk into three segments
    with core-uniform tile capacities, laid out [L-all | RA-all | RB-all]
    in tile-seq space:
      L : src in own shard          -> gather from m1 / m2 (no cc dep)
      RA: remote src with l < 1280  -> layer1: full1 (cc1);
                                       layer2: full2[:NA2] (cc2a)
      RB: remote src with l >= 1280 -> layer1: full1 (cc1);
                                       layer2: full2 (cc2b)
    Overflow always demotes toward RB (cc2b completion implies cc2a on
    the ordered collective stream, so RB tiles may hold A-half rows);
    underflow pads with (row 0, w 0).
  - z2 rows are written to m2 (unified, for local gathers/self rows) and
    to m2A/m2B halves; cc2a fires after z2 chunks 0-9 (mid layer-1
    remote pass), cc2b after chunk 19.
  - Aggregation per layer: psum passes (local incl self+bias, then
    remote) accumulating via SBUF accumulators; layer 2 runs local / RA
    / RB passes so RA work only waits on cc2a.
  - One-hot weight matrices (lhsT of aggregation matmuls) are built on
    device in groups of 16 tiles with two broadcast DVE tensor_tensor
    ops: (iota == dslot_bcast) * w_bcast, from [128, net] bf16 arrays.
  - Gather/collective instructions carry scheduler-only ordering edges
    so the in-order GpSimd engine never blocks on a later collective
    while earlier gather work is pending; RA/RB gather emission is
    interleaved by chunk coverage.
  - Degrees come from a single DVE reduce over a compact host layout of
    the own-shard edge weights (w at [dst%128, dst//128, k]).

Host-side preprocessing is index manipulation / data layout only.  All
arithmetic (degree sums, rsqrt, matmuls, aggregation) runs on device.
"""

import os

import numpy as np
import ml_dtypes

import concourse.bacc as bacc
import concourse.bass as bass
import concourse.mybir as mybir
import concourse.tile as tile
from concourse.bass_utils import run_bass_kernel_spmd
from concourse.masks import make_identity

F32 = mybir.dt.float32
BF16 = mybir.dt.bfloat16
I16 = mybir.dt.int16

N = 20000
E = 320000
IN_FT, HID1, HID2, OUT_FT = 256, 400, 200, 128
NCORES = 8
SHARD = N // NCORES            # 2500 nodes per core
NCH = (SHARD + 127) // 128     # 20 local dst chunks per core (last 68)
TAB1_W = 256                   # padded row width of layer-1 gather table
TL = 2                         # local-segment tile capacity per chunk
H1CH = 10                      # chunks in the z2 A-half (AG2 split)
H1 = H1CH * 128                # 1280
H2 = SHARD - H1                # 1220
NA2 = NCORES * H1              # A-region rows of full2
AluOp = mybir.AluOpType
ActFn = mybir.ActivationFunctionType


def _cdiv(a, b):
    return (a + b - 1) // b


# --------------------------------------------------------------------------
# Host-side sharding / layout
# --------------------------------------------------------------------------

def _idx_layout(a):
    g = a.astype(np.int16).reshape(-1, 16).T.copy()
    return np.ascontiguousarray(np.tile(g, (8, 1)))


def _prep_edges(edge_index, edge_attr):
    """Partition edges by dst shard, pack per-chunk into [L|RA|RB]
    segments with core-uniform tile capacities.  Self loops are NOT in
    the edge lists (identity-stationary on the zself chunk rows)."""
    src = np.ascontiguousarray(edge_index[0]).astype(np.int64)
    dst = np.ascontiguousarray(edge_index[1]).astype(np.int64)
    w_all = np.ascontiguousarray(edge_attr).astype(np.float32)

    per_core = []
    for c in range(NCORES):
        lo = c * SHARD
        m = (dst >= lo) & (dst < lo + SHARD)
        s = src[m]
        d = dst[m] - lo
        w = w_all[m]
        own = (s // SHARD) == c
        ch = d >> 7
        per_core.append((s, d, w, own, ch))

    # capacities (uniform across cores)
    cntRA = np.zeros((NCORES, NCH), np.int64)
    loadB0 = np.zeros((NCORES, NCH), np.int64)
    for c in range(NCORES):
        s, d, w, own, ch = per_core[c]
        l = s % SHARD
        for j in range(NCH):
            mj = ch == j
            ovL = max(0, int((mj & own).sum()) - 128 * TL)
            cntRA[c, j] = int((mj & ~own & (l < H1)).sum())
            loadB0[c, j] = int((mj & ~own & (l >= H1)).sum()) + ovL
    TRA = np.zeros(NCH, np.int64)
    TRB = np.zeros(NCH, np.int64)
    for j in range(NCH):
        TRA[j] = max(1, int(np.round(cntRA[:, j].mean() / 128.0)))
        ovfA = np.maximum(0, cntRA[:, j] - 128 * TRA[j])
        TRB[j] = max(1, int(_cdiv(int((loadB0[:, j] + ovfA).max()), 128)))
    sumTRA, sumTRB = int(TRA.sum()), int(TRB.sum())
    baseRA = NCH * TL
    baseRB = baseRA + sumTRA
    net = baseRB + sumTRB
    offL = [TL * j for j in range(NCH)]
    cumA = np.concatenate([[0], np.cumsum(TRA)])
    cumB = np.concatenate([[0], np.cumsum(TRB)])
    offRA = [baseRA + int(cumA[j]) for j in range(NCH)]
    offRB = [baseRB + int(cumB[j]) for j in range(NCH)]

    # per-core own-degree layout sized by the global max per-dst count
    K2 = 0
    for c in range(NCORES):
        lo = c * SHARD
        m = (dst >= lo) & (dst < lo + SHARD)
        dl = dst[m] - lo
        cc = np.zeros((NCH, 128), np.int64)
        np.add.at(cc, (dl >> 7, dl & 127), 1)
        K2 = max(K2, int(cc.max()))

    in_edges = []
    bf = ml_dtypes.bfloat16
    for c in range(NCORES):
        s, d, w, own, ch = per_core[c]
        l = s % SHARD
        r = s // SHARD
        row2 = np.where(l < H1, r * H1 + l, NA2 + r * H2 + (l - H1))
        erow = np.zeros(net * 128, np.int64)    # layer-1 gather rows
        erow2 = np.zeros(net * 128, np.int64)   # layer-2 gather rows
        dslot = np.zeros(net * 128, np.int64)
        wslot = np.zeros(net * 128, np.float32)

        def fill(seq0, ntile, idx, rows1, rows2):
            o = 128 * seq0
            k = len(idx)
            assert k <= 128 * ntile, (k, ntile)
            erow[o:o + k] = rows1
            erow2[o:o + k] = rows2
            dslot[o:o + k] = d[idx] & 127
            wslot[o:o + k] = w[idx]

        for j in range(NCH):
            mj = ch == j
            iL = np.flatnonzero(mj & own)
            keepL, ovL = iL[:128 * TL], iL[128 * TL:]
            fill(offL[j], TL, keepL, l[keepL], l[keepL])
            iRA = np.flatnonzero(mj & ~own & (l < H1))
            keepA, ovA = iRA[:128 * TRA[j]], iRA[128 * TRA[j]:]
            fill(offRA[j], int(TRA[j]), keepA, s[keepA], row2[keepA])
            iRB = np.concatenate(
                [np.flatnonzero(mj & ~own & (l >= H1)), ovL, ovA])
            fill(offRB[j], int(TRB[j]), iRB, s[iRB], row2[iRB])

        degw_own = np.zeros((128, NCH, K2), np.float32)
        lo = c * SHARD
        m = (dst >= lo) & (dst < lo + SHARD)
        dl_all = dst[m] - lo
        wl_all = w_all[m]
        kfill2 = np.zeros((NCH, 128), np.int64)
        lp, lc = dl_all & 127, dl_all >> 7
        for i in range(len(dl_all)):
            p, chn = int(lp[i]), int(lc[i])
            degw_own[p, chn, kfill2[chn, p]] = wl_all[i]
            kfill2[chn, p] += 1

        in_edges.append({
            "egidx": _idx_layout(erow),
            "egidx2": _idx_layout(erow2),
            "dslotb": np.ascontiguousarray(
                dslot.reshape(net, 128).T).astype(bf),
            "wb": np.ascontiguousarray(
                wslot.reshape(net, 128).T).astype(bf),
            "degw_own": degw_own.reshape(128, -1),
        })
    meta = dict(TRA=[int(x) for x in TRA], TRB=[int(x) for x in TRB],
                offL=offL, offRA=offRA, offRB=offRB,
                baseRA=baseRA, baseRB=baseRB, net=net, K2=K2)
    return meta, in_edges


# --------------------------------------------------------------------------
# Device program
# --------------------------------------------------------------------------

def _fix_multiwait(nc):
    """This neuronxcc build only accepts ONE sync-wait on non-EventSemaphore
    instructions; bacc's splitter allows two on DMAs.  Move excess waits onto
    inserted EventSemaphore NOPs (2 waits each) preceding the instruction."""
    nev = 0
    for bb in nc.main_func.blocks:
        changed = False
        out = []
        for ins in bb.instructions:
            si = ins.sync_info
            waits = list(si.on_wait) if si and si.on_wait else []
            limit = 2 if isinstance(ins, mybir.InstEventSemaphore) else 1
            if len(waits) > limit:
                extra, keep = waits[:-limit], waits[-limit:]
                for i in range(0, len(extra), 2):
                    ev = mybir.InstEventSemaphore(
                        name=f"{ins.name}-evw{i}", ins=[], outs=[])
                    ev.engine = ins.engine
                    ev.sync_info = mybir.SyncInfo(
                        on_wait=extra[i:i + 2], on_update=[])
                    out.append(ev)
                    nev += 1
                si.on_wait = keep
                changed = True
            out.append(ins)
        if changed:
            bb.instructions = out
    return nev


def _dummy_out(nc, wpool, out_d):
    for j in range(NCH):
        cw = min(128, SHARD - 128 * j)
        o_sb = wpool.tile([128, OUT_FT], F32, tag="osb")
        nc.vector.memset(o_sb[:], 0.0)
        nc.sync.dma_start(out=out_d[128 * j:128 * j + cw, :],
                          in_=o_sb[:cw, :])


def build_nc(meta):
    stage = int(os.environ.get("K_STAGE", "500"))
    TRA, TRB = meta["TRA"], meta["TRB"]
    offL, offRA, offRB = meta["offL"], meta["offRA"], meta["offRB"]
    baseRA, baseRB = meta["baseRA"], meta["baseRB"]
    net, K2 = meta["net"], meta["K2"]
    nc = bacc.Bacc("TRN2", target_bir_lowering=False, debug=False,
                   num_devices=NCORES, num_swdge_queues=4)

    xt_d = nc.dram_tensor("xt", [IN_FT, SHARD], BF16, kind="ExternalInput")
    egidx_d = nc.dram_tensor("egidx", [128, 8 * net], I16,
                             kind="ExternalInput")
    egidx2_d = nc.dram_tensor("egidx2", [128, 8 * net], I16,
                              kind="ExternalInput")
    dslotb_d = nc.dram_tensor("dslotb", [128, net], BF16,
                              kind="ExternalInput")
    wb_d = nc.dram_tensor("wb", [128, net], BF16, kind="ExternalInput")
    iotab_d = nc.dram_tensor("iotab", [128, 16 * 128], BF16,
                             kind="ExternalInput")
    degwo_d = nc.dram_tensor("degw_own", [128, NCH * K2], F32,
                             kind="ExternalInput")
    fcw_d = nc.dram_tensor("fcw", [IN_FT, HID1], BF16,
                           kind="ExternalInput")
    fcb_d = nc.dram_tensor("fcb", [HID1, 1], F32, kind="ExternalInput")
    w1_d = nc.dram_tensor("w1", [HID1, HID2], BF16, kind="ExternalInput")
    b1_d = nc.dram_tensor("b1", [1, HID2], BF16, kind="ExternalInput")
    w2_d = nc.dram_tensor("w2", [HID2, OUT_FT], BF16,
                          kind="ExternalInput")
    b2_d = nc.dram_tensor("b2", [1, OUT_FT], BF16, kind="ExternalInput")
    out_d = nc.dram_tensor("out", [SHARD, OUT_FT], F32,
                           kind="ExternalOutput")

    n_fi = _cdiv(IN_FT, 128)     # 2
    n_fo = _cdiv(HID1, 128)      # 4 (128,128,128,16)
    n_k2 = _cdiv(HID2, 128)      # 2 (128,72)
    fo_sizes = [min(128, HID1 - 128 * i) for i in range(n_fo)]
    k2_sizes = [min(128, HID2 - 128 * i) for i in range(n_k2)]
    NSUB = 5
    SUB = SHARD // NSUB          # 500

    def _emit(tc, cpool, gpool, wpool, apool, psA, psB, psT, dpool):
        # ---------------- early inputs ----------------
        degwo_sb = cpool.tile([128, NCH * K2], F32)
        nc.sync.dma_start(out=degwo_sb[:], in_=degwo_d[:])
        egidx_sb = cpool.tile([128, 8 * net], I16)
        nc.sync.dma_start(out=egidx_sb[:], in_=egidx_d[:])
        egidx2_sb = cpool.tile([128, 8 * net], I16)
        nc.sync.dma_start(out=egidx2_sb[:], in_=egidx2_d[:])
        dslot_sb = cpool.tile([128, net], BF16)
        nc.sync.dma_start(out=dslot_sb[:], in_=dslotb_d[:])
        wb_sb = cpool.tile([128, net], BF16)
        nc.sync.dma_start(out=wb_sb[:], in_=wb_d[:])
        iota_sb = cpool.tile([128, 16 * 128], BF16)
        nc.sync.dma_start(out=iota_sb[:], in_=iotab_d[:])
        fcb_sb = cpool.tile([128, n_fo], F32, name="fcb_sb")
        for i in range(n_fo):
            nc.sync.dma_start(
                out=fcb_sb[:fo_sizes[i], i:i + 1],
                in_=fcb_d[128 * i:128 * i + fo_sizes[i], :])

        fcw_sb = []
        for i in range(n_fi):
            t = cpool.tile([128, HID1], BF16, name=f"fcw{i}")
            nc.scalar.dma_start(out=t[:],
                                in_=fcw_d[128 * i:128 * (i + 1), :])
            fcw_sb.append(t)
        w1_sb = []
        for i in range(n_fo):
            t = cpool.tile([fo_sizes[i], HID2], BF16, name=f"w1_{i}")
            nc.scalar.dma_start(
                out=t[:], in_=w1_d[128 * i:128 * i + fo_sizes[i], :])
            w1_sb.append(t)
        w2_sb = []
        for i in range(n_k2):
            t = cpool.tile([k2_sizes[i], OUT_FT], BF16, name=f"w2_{i}")
            nc.scalar.dma_start(
                out=t[:], in_=w2_d[128 * i:128 * i + k2_sizes[i], :])
            w2_sb.append(t)
        b1_sb = cpool.tile([1, HID2], BF16)
        nc.scalar.dma_start(out=b1_sb[:], in_=b1_d[:])
        b2_sb = cpool.tile([1, OUT_FT], BF16)
        nc.scalar.dma_start(out=b2_sb[:], in_=b2_d[:])

        ident = cpool.tile([128, 128], BF16)
        make_identity(nc, ident[:])

        # ---------------- degrees / normalization (DVE) ---------------
        deg_own = cpool.tile([128, NCH], F32)
        nc.vector.tensor_reduce(
            out=deg_own[:],
            in_=degwo_sb[:].rearrange("p (c k) -> p c k", k=K2),
            axis=mybir.AxisListType.X, op=AluOp.add)
        nc.vector.tensor_scalar_add(deg_own[:], deg_own[:], 1.0)
        dinv_own = cpool.tile([128, NCH], F32)
        nc.vector.reciprocal(out=dinv_own[:], in_=deg_own[:])
        nc.scalar.activation(out=dinv_own[:], in_=dinv_own[:],
                             func=ActFn.Sqrt)
        sqd_own = cpool.tile([128, NCH], BF16)
        nc.scalar.activation(out=sqd_own[:], in_=deg_own[:],
                             func=ActFn.Sqrt)

        m1 = dpool.tile([SHARD, TAB1_W], BF16)
        m2 = dpool.tile([SHARD, OUT_FT], BF16)
        m2A = dpool.tile([H1, OUT_FT], BF16)
        m2B = dpool.tile([H2, OUT_FT], BF16)

        if stage < 10:
            _dummy_out(nc, wpool, out_d)
            return

        # ---------------- phase A: z1 (own shard) -> m1 ---------------
        with tc.tile_pool(name="phA", bufs=1) as ppool:
            h0strip = []
            for i in range(n_fo):
                t_h = ppool.tile([fo_sizes[i], SHARD], BF16,
                                 name=f"h0strip{i}")
                h0strip.append(t_h)
            with tc.tile_pool(name="phAw", bufs=2) as tpool:
                nxt_ch = 0
                for s in range(NSUB):
                    xts = []
                    for k in range(n_fi):
                        xk = tpool.tile([128, SUB], BF16, tag="xts",
                                        name=f"xts{k}", bufs=3)
                        nc.scalar.dma_start(
                            out=xk[:],
                            in_=xt_d[128 * k:128 * (k + 1),
                                     SUB * s:SUB * (s + 1)])
                        xts.append(xk)
                    for i in range(n_fo):
                        ps_h = psA.tile([fo_sizes[i], SUB], F32, tag="ph")
                        for k in range(n_fi):
                            nc.tensor.matmul(
                                out=ps_h[:],
                                lhsT=fcw_sb[k][:, 128 * i:128 * i
                                               + fo_sizes[i]],
                                rhs=xts[k][:],
                                start=(k == 0), stop=(k == n_fi - 1),
                            )
                        nc.vector.tensor_scalar(
                            out=h0strip[i][:, SUB * s:SUB * (s + 1)],
                            in0=ps_h[:],
                            scalar1=fcb_sb[:fo_sizes[i], i:i + 1],
                            scalar2=0.0,
                            op0=AluOp.add, op1=AluOp.max,
                        )
                    end = SUB * (s + 1)
                    while (nxt_ch + 1) * 128 <= end or (
                            s == NSUB - 1 and nxt_ch < NCH):
                        ch = nxt_ch
                        nxt_ch += 1
                        cw = min(128, SHARD - 128 * ch)
                        ps_z = psB.tile([128, HID2], F32, tag="b")
                        for i in range(n_fo):
                            nc.tensor.matmul(
                                out=ps_z[:cw, :],
                                lhsT=h0strip[i][:, 128 * ch:128 * ch + cw],
                                rhs=w1_sb[i][:],
                                start=(i == 0), stop=(i == n_fo - 1),
                            )
                        zrow = tpool.tile([128, TAB1_W], BF16, tag="zrow",
                                          name="zrow", bufs=3)
                        nc.scalar.mul(out=zrow[:cw, :HID2],
                                      in_=ps_z[:cw, :],
                                      mul=dinv_own[:cw, ch:ch + 1])
                        nc.sync.dma_start(
                            out=m1[128 * ch:128 * ch + cw, :],
                            in_=zrow[:cw, :])

        if stage < 12:
            _dummy_out(nc, wpool, out_d)
            return

        # ---------------- collective: layer-1 table -------------------
        rg = [list(range(NCORES))]
        full1 = nc.dram_tensor("full1", [N, TAB1_W], BF16,
                               addr_space="Shared")
        cc1_i = nc.gpsimd.collective_compute(
            "AllGather", AluOp.bypass, replica_groups=rg,
            ins=[m1.opt()], outs=[full1.ap()[:]],
        )
        cc1 = [cc1_i.ins]
        gps_chain = [cc1_i.ins]

        # deferred: sqd row layout (PE op; avoid head-of-line pre-phA)
        ps_tr = psT.tile([NCH, 128], BF16, tag="tr")
        nc.tensor.transpose(out=ps_tr[:], in_=sqd_own[:],
                            identity=ident[:])
        sqd_rows = cpool.tile([NCH, 128], BF16)
        nc.vector.tensor_copy(out=sqd_rows[:], in_=ps_tr[:])
        sqdT = cpool.tile([1, 128 * NCH], BF16)
        for j in range(NCH):
            nc.sync.dma_start(out=sqdT[:, 128 * j:128 * (j + 1)],
                              in_=sqd_rows[j:j + 1, :])

        if stage < 14:
            _dummy_out(nc, wpool, out_d)
            return

        # ---------------- gather emission helper ----------------------
        gq = [0]

        def emit_one_gather(streams, tag, table, width, ccdeps, idx_sb,
                            k, nt, grain, bufs):
            graw = gpool.tile([128, grain * width], BF16, tag=tag,
                              name=f"g{tag}", bufs=bufs)
            sub = graw[:, :nt * width].rearrange("p (t f) -> p t f",
                                                 f=width)
            if isinstance(table, bass.DRamTensorHandle):
                table_ap = table.ap()
            else:
                table_ap = table[:]
            gi = nc.gpsimd.dma_gather(
                sub, table_ap, idx_sb[:, 8 * k:8 * (k + nt)],
                nt * 128, nt * 128, width, queue_num=gq[0] % 4)
            gq[0] += 1
            for cc in ccdeps:
                tile.add_dep_helper(gi.ins, cc,
                                    reason="gather reads AG table")
            if gps_chain:
                tile.add_dep_helper(gi.ins, gps_chain[-1], sync=False,
                                    reason="gpsimd issue order")
            gps_chain.append(gi.ins)
            for t in range(nt):
                streams[k + t] = (graw, t)

        def emit_gathers(streams, tag, table, width, ccdeps, seqs,
                         grain, bufs, idx_sb):
            s0, s1 = seqs
            k = s0
            while k < s1:
                nt = min(grain, s1 - k)
                emit_one_gather(streams, tag, table, width, ccdeps,
                                idx_sb, k, nt, grain, bufs)
                k += nt

        def emit_interleaved(streams, tagA, tagB, table, width, ccA, ccB,
                             grain, bufsA, bufsB, idx_sb):
            """Emit RA-run and RB-run gathers interleaved by chunk
            coverage so per-chunk consumption (RA_j then RB_j) is fed
            evenly."""
            kA, kB = baseRA, baseRB
            # chunk covered by next emission, per run
            def chunkA():
                return np.searchsorted(np.cumsum(TRA), kA - baseRA,
                                       side='right')
            def chunkB():
                return np.searchsorted(np.cumsum(TRB), kB - baseRB,
                                       side='right')
            while kA < baseRB or kB < net:
                if kB >= net or (kA < baseRB and chunkA() <= chunkB()):
                    nt = min(grain, baseRB - kA)
                    emit_one_gather(streams, tagA, table, width, ccA,
                                    idx_sb, kA, nt, grain, bufsA)
                    kA += nt
                else:
                    nt = min(grain, net - kB)
                    emit_one_gather(streams, tagB, table, width, ccB,
                                    idx_sb, kB, nt, grain, bufsB)
                    kB += nt

        # one-hot weight tiles: built on DVE in groups of 16 via
        # broadcast tensor_tensor; lazily, just before first use so the
        # DVE stream interleaves with the pass ops (release order).
        sw_groups = {}

        def swtile(seq, region0, region1):
            g0 = region0 + ((seq - region0) // 16) * 16
            if g0 not in sw_groups:
                nt = min(16, region1 - g0)
                swg = wpool.tile([128, 16 * 128], BF16, tag="sw",
                                 bufs=8)
                eq = wpool.tile([128, 16 * 128], BF16, tag="sweq",
                                bufs=1)
                e3 = eq[:, :nt * 128].rearrange("p (t f) -> p t f",
                                                f=128)
                nc.vector.tensor_tensor(
                    out=e3,
                    in0=iota_sb[:, :nt * 128].rearrange(
                        "p (t f) -> p t f", f=128),
                    in1=dslot_sb[:, g0:g0 + nt].broadcast_to(
                        [128, nt, 128]),
                    op=AluOp.is_equal)
                nc.vector.tensor_tensor(
                    out=swg[:, :nt * 128].rearrange(
                        "p (t f) -> p t f", f=128),
                    in0=e3,
                    in1=wb_sb[:, g0:g0 + nt].broadcast_to(
                        [128, nt, 128]),
                    op=AluOp.mult)
                sw_groups[g0] = swg
            return sw_groups[g0][:, (seq - g0) * 128:
                                 (seq - g0 + 1) * 128]

        def agg_mm(ps, g, seq, width, start, stop, region):
            graw, t = g[seq]
            sw = swtile(seq, region[0], region[1])
            nc.tensor.matmul(
                out=ps[:],
                lhsT=sw,
                rhs=graw[:, t * width:t * width + (HID2 if width == TAB1_W
                                                   else width)],
                start=start, stop=stop,
            )

        # ---------------- layer-1 gathers (gpsimd order) --------------
        g1 = {}
        emit_gathers(g1, "g1L", m1, TAB1_W, [], (0, baseRA), 8, 3,
                     egidx_sb)
        if stage >= 250:
            emit_interleaved(g1, "g1RA", "g1RB", full1, TAB1_W, cc1,
                             cc1, 8, 4, 6, egidx_sb)

        # ---------------- layer-1 passes ------------------------------
        l1acc = apool.tile([128, NCH, HID2], F32)
        # local pass: L + self + bias
        for j in range(NCH):
            cw = min(128, SHARD - 128 * j)
            zself = wpool.tile([128, TAB1_W], BF16, tag="zself1", bufs=2)
            nc.sync.dma_start(out=zself[:cw, :],
                              in_=m1[128 * j:128 * j + cw, :])
            ps = psB.tile([128, HID2], F32, tag="b")
            for t in range(TL):
                agg_mm(ps, g1, offL[j] + t, TAB1_W, t == 0, False,
                       (0, baseRA))
            nc.tensor.matmul(out=ps[:], lhsT=ident[:cw, :],
                             rhs=zself[:cw, :HID2],
                             start=False, stop=False)
            nc.tensor.matmul(out=ps[:],
                             lhsT=sqdT[:, 128 * j:128 * (j + 1)],
                             rhs=b1_sb[:], start=False, stop=True)
            nc.vector.tensor_copy(out=l1acc[:, j, :], in_=ps[:])

        if stage < 250:
            _dummy_out(nc, wpool, out_d)
            return

        # remote pass (RA_j + RB_j in one psum group) + z2 production
        full2 = nc.dram_tensor("full2", [N, OUT_FT], BF16,
                               addr_space="Shared")
        cc2a_holder = []
        for j in range(NCH):
            cw = min(128, SHARD - 128 * j)
            ps = psB.tile([128, HID2], F32, tag="b")
            nmm = TRA[j] + TRB[j]
            k = 0
            for t in range(TRA[j]):
                agg_mm(ps, g1, offRA[j] + t, TAB1_W, k == 0,
                       k == nmm - 1, (baseRA, baseRB))
                k += 1
            for t in range(TRB[j]):
                agg_mm(ps, g1, offRB[j] + t, TAB1_W, k == 0,
                       k == nmm - 1, (baseRB, net))
                k += 1
            acc = wpool.tile([128, HID2], F32, tag="l1f", bufs=2)
            nc.vector.tensor_tensor(out=acc[:], in0=ps[:],
                                    in1=l1acc[:, j, :], op=AluOp.add)
            l1row = wpool.tile([128, HID2], BF16, tag="l1r", bufs=2)
            nc.scalar.activation(out=l1row[:], in_=acc[:],
                                 func=ActFn.Relu,
                                 scale=dinv_own[:, j:j + 1])
            # ---- z2 for chunk j ----
            l1T = []
            for i in range(n_k2):
                ps_tr2 = psT.tile([128, 128], BF16, tag="tr")
                nc.tensor.transpose(
                    out=ps_tr2[:k2_sizes[i], :],
                    in_=l1row[:, 128 * i:128 * i + k2_sizes[i]],
                    identity=ident[:],
                )
                lt2 = wpool.tile([128, 128], BF16, tag="l1T")
                nc.vector.tensor_copy(out=lt2[:k2_sizes[i], :],
                                      in_=ps_tr2[:k2_sizes[i], :])
                l1T.append(lt2)
            ps_z2 = psB.tile([128, OUT_FT], F32, tag="b")
            for i in range(n_k2):
                nc.tensor.matmul(
                    out=ps_z2[:],
                    lhsT=l1T[i][:k2_sizes[i], :],
                    rhs=w2_sb[i][:],
                    start=(i == 0), stop=(i == n_k2 - 1),
                )
            zrow2 = wpool.tile([128, OUT_FT], BF16, tag="zrow2", bufs=3)
            nc.scalar.mul(out=zrow2[:], in_=ps_z2[:],
                          mul=dinv_own[:, j:j + 1])
            nc.sync.dma_start(out=m2[128 * j:128 * j + cw, :],
                              in_=zrow2[:cw, :])
            if j < H1CH:
                nc.sync.dma_start(out=m2A[128 * j:128 * j + cw, :],
                                  in_=zrow2[:cw, :])
            else:
                o = 128 * j - H1
                nc.sync.dma_start(out=m2B[o:o + cw, :],
                                  in_=zrow2[:cw, :])
            if j == H1CH - 1 and stage >= 400:
                cc2a_i = nc.gpsimd.collective_compute(
                    "AllGather", AluOp.bypass, replica_groups=rg,
                    ins=[m2A.opt()], outs=[full2.ap()[0:NA2, :]],
                )
                tile.add_dep_helper(cc2a_i.ins, gps_chain[-1],
                                    sync=False,
                                    reason="gpsimd issue order")
                gps_chain.append(cc2a_i.ins)
                cc2a_holder.append(cc2a_i.ins)

        if stage < 400:
            for j in range(NCH):
                cw = min(128, SHARD - 128 * j)
                o_sb = wpool.tile([128, OUT_FT], F32, tag="osb")
                nc.scalar.copy(out=o_sb[:],
                               in_=l1acc[:, j, :OUT_FT])
                nc.sync.dma_start(out=out_d[128 * j:128 * j + cw, :],
                                  in_=o_sb[:cw, :])
            return

        cc2a = [cc2a_holder[0]]
        cc2b_i = nc.gpsimd.collective_compute(
            "AllGather", AluOp.bypass, replica_groups=rg,
            ins=[m2B.opt()], outs=[full2.ap()[NA2:N, :]],
        )
        cc2b = [cc2b_i.ins]
        tile.add_dep_helper(cc2b_i.ins, gps_chain[-1], sync=False,
                            reason="gpsimd issue order")
        gps_chain.append(cc2b_i.ins)

        if stage < 500:
            _dummy_out(nc, wpool, out_d)
            return

        # ---------------- layer-2 gathers + passes --------------------
        g2 = {}
        emit_gathers(g2, "g2L", m2, OUT_FT, [], (0, baseRA), 8, 3,
                     egidx2_sb)
        emit_gathers(g2, "g2RA", full2, OUT_FT, cc2a, (baseRA, baseRB),
                     8, 5, egidx2_sb)
        emit_gathers(g2, "g2RB", full2, OUT_FT, cc2b, (baseRB, net),
                     8, 6, egidx2_sb)

        sw_groups.clear()
        l2acc = apool.tile([128, NCH, OUT_FT], BF16)
        for j in range(NCH):
            cw = min(128, SHARD - 128 * j)
            zself = wpool.tile([128, OUT_FT], BF16, tag="zself2", bufs=2)
            nc.sync.dma_start(out=zself[:cw, :],
                              in_=m2[128 * j:128 * j + cw, :])
            ps = psB.tile([128, OUT_FT], F32, tag="b")
            for t in range(TL):
                agg_mm(ps, g2, offL[j] + t, OUT_FT, t == 0, False,
                       (0, baseRA))
            nc.tensor.matmul(out=ps[:], lhsT=ident[:cw, :],
                             rhs=zself[:cw, :],
                             start=False, stop=False)
            nc.tensor.matmul(out=ps[:],
                             lhsT=sqdT[:, 128 * j:128 * (j + 1)],
                             rhs=b2_sb[:], start=False, stop=True)
            nc.vector.tensor_copy(out=l2acc[:, j, :], in_=ps[:])
        # RA pass
        for j in range(NCH):
            ps = psB.tile([128, OUT_FT], F32, tag="b")
            for t in range(TRA[j]):
                agg_mm(ps, g2, offRA[j] + t, OUT_FT, t == 0,
                       t == TRA[j] - 1, (baseRA, baseRB))
            nc.vector.tensor_tensor(out=l2acc[:, j, :], in0=ps[:],
                                    in1=l2acc[:, j, :], op=AluOp.add)
        # RB pass + output
        for j in range(NCH):
            cw = min(128, SHARD - 128 * j)
            ps = psB.tile([128, OUT_FT], F32, tag="b")
            for t in range(TRB[j]):
                agg_mm(ps, g2, offRB[j] + t, OUT_FT, t == 0,
                       t == TRB[j] - 1, (baseRB, net))
            o_f32 = wpool.tile([128, OUT_FT], F32, tag="of")
            nc.vector.tensor_tensor(out=o_f32[:], in0=ps[:],
                                    in1=l2acc[:, j, :], op=AluOp.add)
            o_sb = wpool.tile([128, OUT_FT], F32, tag="osb")
            nc.scalar.activation(out=o_sb[:], in_=o_f32[:],
                                 func=ActFn.Relu,
                                 scale=dinv_own[:, j:j + 1])
            nc.sync.dma_start(out=out_d[128 * j:128 * j + cw, :],
                              in_=o_sb[:cw, :])

    with tile.TileContext(nc) as tc:
        with (
            tc.tile_pool(name="const", bufs=1) as cpool,
            tc.tile_pool(name="gath", bufs=1) as gpool,
            tc.tile_pool(name="work", bufs=2) as wpool,
            tc.tile_pool(name="acc", bufs=1) as apool,
            tc.tile_pool(name="psA", bufs=3, space="PSUM") as psA,
            tc.tile_pool(name="psB", bufs=3, space="PSUM") as psB,
            tc.tile_pool(name="psT", bufs=2, space="PSUM") as psT,
            tc.tile_pool(name="dram", bufs=1, space="DRAM") as dpool,
        ):
            _emit(tc, cpool, gpool, wpool, apool, psA, psB, psT, dpool)
    nc.compile()
    _fix_multiwait(nc)
    return nc


# --------------------------------------------------------------------------
# Entry point
# --------------------------------------------------------------------------

_NC_CACHE = {}


def kernel(x, edge_index, edge_attr, fc_W, fc_b, W1, b1, W2, b2,
           _trace=False):
    meta, in_edges = _prep_edges(edge_index, edge_attr)
    key = (tuple(meta["TRA"]), tuple(meta["TRB"]), meta["K2"])
    if key not in _NC_CACHE:
        _NC_CACHE[key] = build_nc(meta)
    nc = _NC_CACHE[key]

    x = np.asarray(x, np.float32)
    bf = ml_dtypes.bfloat16
    iotab = np.ascontiguousarray(
        np.tile(np.arange(128, dtype=np.float32), (128, 16))).astype(bf)
    shared = {
        "fcw": np.asarray(fc_W, np.float32).astype(bf),
        "fcb": np.asarray(fc_b, np.float32).reshape(HID1, 1),
        "w1": np.asarray(W1, np.float32).astype(bf),
        "b1": np.asarray(b1, np.float32).reshape(1, HID2).astype(bf),
        "w2": np.asarray(W2, np.float32).astype(bf),
        "b2": np.asarray(b2, np.float32).reshape(1, OUT_FT).astype(bf),
        "iotab": iotab,
    }
    in_maps = []
    for c in range(NCORES):
        xt = np.ascontiguousarray(
            x[c * SHARD:(c + 1) * SHARD, :].T).astype(bf)
        in_maps.append({"xt": xt, **in_edges[c], **shared})

    res = run_bass_kernel_spmd(nc, in_maps, list(range(NCORES)),
                               trace=_trace)
    out = np.concatenate([res.results[c]["out"] for c in range(NCORES)],
                         axis=0)
    if _trace:
        kernel._last_exec_time_ns = res.exec_time_ns
        kernel._last_results = res
    return out
